# revision 1
# baseline (speedup 1.0000x reference)
"""Trainium2 Bass kernel for a LongNet attention block.

Problem: x (1,48,256,256) -> patchify to 16384 tokens of dim 192 ->
4 segments of 4096 tokens -> q/k/v proj + LayerNorm each -> full
attention within each segment -> un-patchify.

Sharding: 2 cores per segment (8 cores, 4 segments). Each core computes
attention for 2048 queries against its segment's full 4096 keys/values.
Softmax is key-order invariant, so the host permutes each core's token
columns so its query half is always columns 0:2048 -> one SPMD program.

Device pipeline per core (matmuls bf16 in / f32 PSUM accumulate):
  1. q/k/v projections from transposed tokens xsT [d, s] with a ones-row
     folding the bias into the matmul. Mean-centering is folded into the
     weights on the host (W' = W - colmean(W)), so projections emerge
     already centered; q+k (or k+v) share one N=384 matmul.
  2. LayerNorm variance via one square-with-accumulate DVE op per
     projection; rsqrt = ScalarE Sqrt + VectorE reciprocal. Sqrt shares
     an activation table set with Copy/Square, Exp has its own ->
     exactly two table loads for the whole kernel.
  3. Normalized q is PE-transposed to [e, s] layout; k is transposed
     UN-scaled and its 1/std is applied later as the exp's per-partition
     scale AP (softmax keys live on partitions there). v stays natural
     with a ones-column appended so the softmax denominator falls out of
     the attention matmul for free.
  4. scoresT = kT.T @ qT per 128-key chunk over query slab-pairs; exp is
     applied by ScalarE straight out of PSUM on [128,1024] tiles with
     scale = r_k/sqrt(D). No row-max subtraction is needed:
     |scores| <= D / sqrt(D) ~= 13.9.
  5. outT[e, sq] accumulates v.T @ p over key chunks in PSUM; the ones
     column of v produces the rowsum row. Host divides and transposes.
"""

import contextlib

import numpy as np
import ml_dtypes

import concourse.bacc as bacc
import concourse.mybir as mybir
import concourse.tile as tile
from concourse.bass_utils import run_bass_kernel_spmd

WS = 2
C = 48
IMG = 256
NS = IMG // WS          # 128
D = C * WS * WS         # 192
S = NS * NS             # 16384
SEG = 4096
G = S // SEG            # 4 segments
NQ = SEG // 2           # 2048 queries per core
NCORES = 8
EPS = 1e-5
SCALE_C = float(D) ** -0.5
SLAB = 512
NKC = SEG // 128        # 32 key chunks
NQC = NQ // 128         # 16 query chunks
VW = 200                # padded per-chunk v width (192 data + ones col @192)

F32 = mybir.dt.float32
BF16 = mybir.dt.bfloat16
FT = mybir.ActivationFunctionType
OP = mybir.AluOpType

_PROGRAM_CACHE = {}

def _build_program(general_gb: bool):
    nc = bacc.Bacc(
        "TRN2",
        target_bir_lowering=False,
        debug=False,
        enable_asserts=False,
    )
    xa = nc.dram_tensor("xa", [128, SEG], BF16, kind="ExternalInput").ap()
    xb = nc.dram_tensor("xb", [128, SEG], BF16, kind="ExternalInput").ap()
    wa = nc.dram_tensor("wa", [128, 3 * D], BF16, kind="ExternalInput").ap()
    wb = nc.dram_tensor("wb", [128, 3 * D], BF16, kind="ExternalInput").ap()
    # k-weight transposed-aug blocks for the direct kT projection
    wka = nc.dram_tensor("wka", [128, D], BF16, kind="ExternalInput").ap()
    wkb = nc.dram_tensor("wkb", [128, D], BF16, kind="ExternalInput").ap()
    idn = nc.dram_tensor("idn", [128, 128], BF16, kind="ExternalInput").ap()
    if general_gb:
        gcol = nc.dram_tensor("gcol", [D, 1], F32, kind="ExternalInput").ap()
        bcol = nc.dram_tensor("bcol", [D, 1], F32, kind="ExternalInput").ap()
        gbc = nc.dram_tensor("gbc", [128, D], F32, kind="ExternalInput").ap()
        bbc = nc.dram_tensor("bbc", [128, D], F32, kind="ExternalInput").ap()
    outa = nc.dram_tensor("outa", [128, NQ], F32, kind="ExternalOutput").ap()
    outb = nc.dram_tensor("outb", [65, NQ], F32, kind="ExternalOutput").ap()

    NSL = NQ // SLAB   # 4 query slabs
    NT = NKC // 4      # 8 key s-tiles of 512

    with tile.TileContext(nc) as tc:
        with contextlib.ExitStack() as stk:
            const = stk.enter_context(tc.tile_pool(name="const", bufs=1))
            persist = stk.enter_context(tc.tile_pool(name="persist", bufs=1))
            ln_sb = stk.enter_context(tc.tile_pool(name="ln_sb", bufs=4))
            smalls = stk.enter_context(tc.tile_pool(name="smalls", bufs=4))
            pt_pool = stk.enter_context(tc.tile_pool(name="pt_pool", bufs=4))
            ev = stk.enter_context(tc.tile_pool(name="ev", bufs=4))

            # input tokens as 8 per-s-tile tiles so section A can start
            # before the whole input has landed (parallel DMA queues)
            xat = [const.tile([128, 512], BF16, name=f"xat{t}")
                   for t in range(NKC // 4)]
            xbt = [const.tile([128, 512], BF16, name=f"xbt{t}")
                   for t in range(NKC // 4)]
            for t in range(NKC // 4):
                tsl = slice(t * 512, (t + 1) * 512)
                nc.sync.dma_start(xat[t], xa[:, tsl])
                nc.sync.dma_start(xbt[t], xb[:, tsl])
            wa_s = const.tile([128, 3 * D], BF16)
            nc.sync.dma_start(wa_s, wa)
            wb_s = const.tile([128, 3 * D], BF16)
            nc.sync.dma_start(wb_s, wb)
            wka_s = const.tile([128, D], BF16)
            nc.sync.dma_start(wka_s, wka)
            wkb_s = const.tile([128, D], BF16)
            nc.sync.dma_start(wkb_s, wkb)
            idn_s = const.tile([128, 128], BF16)
            nc.sync.dma_start(idn_s, idn)
            epsc = const.tile([128, 1], F32)
            nc.gpsimd.memset(epsc, EPS)
            halfc = const.tile([128, 1], F32)
            nc.gpsimd.memset(halfc, 0.5)
            if general_gb:
                gca = const.tile([128, 1], F32)
                nc.sync.dma_start(gca, gcol[0:128])
                gcb = const.tile([64, 1], F32)
                nc.sync.dma_start(gcb, gcol[128:192])
                bca = const.tile([128, 1], F32)
                nc.sync.dma_start(bca, bcol[0:128])
                bcb = const.tile([64, 1], F32)
                nc.sync.dma_start(bcb, bcol[128:192])
                gbc_s = const.tile([128, D], F32)
                nc.sync.dma_start(gbc_s, gbc)
                bbc_s = const.tile([128, D], F32)
                nc.sync.dma_start(bbc_s, bbc)

            # persistent state
            qT0s = [persist.tile([128, SLAB], BF16, name=f"qT0s{s}")
                    for s in range(NSL)]
            qT1s = [persist.tile([128, SLAB], BF16, name=f"qT1s{s}")
                    for s in range(NSL)]
            kT0t = [persist.tile([128, 512], BF16, name=f"kT0t{t}")
                    for t in range(NT)]
            kT1t = [persist.tile([128, 512], BF16, name=f"kT1t{t}")
                    for t in range(NT)]
            vatc = [persist.tile([128, VW], BF16, name=f"vatc{c}")
                    for c in range(NKC)]
            cpreQ = [persist.tile([128, D], BF16, name=f"cpreQ{c}")
                     for c in range(NQC)]
            cpreV = [persist.tile([128, D], BF16, name=f"cpreV{c}")
                     for c in range(NKC)]
            cpreK = ([persist.tile([128, D], BF16, name=f"cpreK{c}")
                      for c in range(NKC)] if general_gb else None)
            ssqQ = persist.tile([128, NQC], F32)
            ssqKV = persist.tile([128, 2 * NKC], F32)  # k: 0:32, v: 32:64
            rQ = persist.tile([128, NQC], F32)
            rKV = persist.tile([128, 2 * NKC], F32)
            rkc = persist.tile([128, NKC], F32)  # SCALE_C / std_k per key
            for s in range(NSL):
                nc.gpsimd.memset(qT1s[s][64:128, :], 0.0)
            for t in range(NT):
                nc.gpsimd.memset(kT1t[t][64:128, :], 0.0)
            for c in range(NKC):
                nc.gpsimd.memset(vatc[c][:, 192:193], 1.0)

            # ---- Section A: all projections, evictions + sum-of-squares ----
            # work items: 16 q chunks then 32 fused k+v chunks; the direct
            # kT projection (no LN needed on that copy) is interleaved.
            def kt_proj(pool, t):
                tsl = slice(t * 512, (t + 1) * 512)
                kp0 = pool.tile([128, 512], F32, name="kp0")
                nc.tensor.matmul(kp0, lhsT=wka_s[:, 0:128], rhs=xat[t],
                                 start=True, stop=False)
                nc.tensor.matmul(kp0, lhsT=wkb_s[:, 0:128], rhs=xbt[t],
                                 start=False, stop=True)
                kp1 = pool.tile([64, 512], F32, name="kp1")
                nc.tensor.matmul(kp1, lhsT=wka_s[:, 128:192], rhs=xat[t],
                                 start=True, stop=False)
                nc.tensor.matmul(kp1, lhsT=wkb_s[:, 128:192], rhs=xbt[t],
                                 start=False, stop=True)
                nc.vector.tensor_copy(kT0t[t], kp0)
                nc.vector.tensor_copy(kT1t[t][0:64, :], kp1)

            with tc.tile_pool(name="pa_raw", bufs=3, space="PSUM") as pa_raw, \
                 tc.tile_pool(name="pa_kt", bufs=1, space="PSUM") as pa_kt:
                for c in range(NQC):
                    jsl = slice((c % 4) * 128, (c % 4 + 1) * 128)
                    raw = pa_raw.tile([128, D], F32, name="rawQ")
                    nc.tensor.matmul(raw, lhsT=xat[c // 4][:, jsl],
                                     rhs=wa_s[:, 0:D], start=True, stop=False)
                    nc.tensor.matmul(raw, lhsT=xbt[c // 4][:, jsl],
                                     rhs=wb_s[:, 0:D], start=False, stop=True)
                    nc.vector.tensor_copy(cpreQ[c], raw)
                    sqd = ln_sb.tile([128, D], BF16, name="sqd")
                    nc.scalar.activation(sqd, raw, FT.Square,
                                         accum_out=ssqQ[:, c:c + 1])
                    if not general_gb and c % 2 == 1:
                        kt_proj(pa_kt, c // 2)
                for c in range(NKC):
                    jsl = slice((c % 4) * 128, (c % 4 + 1) * 128)
                    raw = pa_raw.tile([128, 2 * D], F32, name="rawKV")
                    nc.tensor.matmul(raw, lhsT=xat[c // 4][:, jsl],
                                     rhs=wa_s[:, D:3 * D],
                                     start=True, stop=False)
                    nc.tensor.matmul(raw, lhsT=xbt[c // 4][:, jsl],
                                     rhs=wb_s[:, D:3 * D],
                                     start=False, stop=True)
                    # k: square straight from PSUM on ScalarE (only stats
                    # are needed; kT comes from the direct projection)
                    sqd = ln_sb.tile([128, D], BF16, name="sqd")
                    nc.scalar.activation(sqd, raw[:, 0:D], FT.Square,
                                         accum_out=ssqKV[:, c:c + 1])
                    if general_gb:
                        nc.vector.tensor_copy(cpreK[c], raw[:, 0:D])
                    # v: evict on VectorE; square alternates between engines
                    nc.vector.tensor_copy(cpreV[c], raw[:, D:2 * D])
                    if c % 2 == 0:
                        sqd2 = ln_sb.tile([128, D], BF16, name="sqd2")
                        nc.vector.scalar_tensor_tensor(
                            sqd2, cpreV[c], 1.0, cpreV[c], OP.mult, OP.mult,
                            accum_out=ssqKV[:, NKC + c:NKC + c + 1])
                    else:
                        sqd2 = ln_sb.tile([128, D], BF16, name="sqd2")
                        nc.scalar.activation(
                            sqd2, raw[:, D:2 * D], FT.Square,
                            accum_out=ssqKV[:, NKC + c:NKC + c + 1])

            # ---- batched rsqrt: r = rsqrt(ssq/D + eps) for all chunks ----
            # guess exp(-0.5(v-1)) then two Newton steps, all on wide tiles
            def batched_r(ssq_t, r_t, w):
                vv = smalls.tile([128, 2 * NKC], F32, name="vv")
                nc.vector.tensor_scalar(vv[:, 0:w], ssq_t[:, 0:w], 1.0 / D,
                                        EPS, OP.mult, OP.add)
                nc.scalar.activation(r_t[:, 0:w], vv[:, 0:w], FT.Exp,
                                     scale=-0.5, bias=halfc)
                hv = smalls.tile([128, 2 * NKC], F32, name="hv")
                nc.vector.tensor_scalar(hv[:, 0:w], vv[:, 0:w], -0.5, None,
                                        OP.mult)
                cur = r_t
                for it in range(2):
                    b = smalls.tile([128, 2 * NKC], F32, name=f"nb{it}")
                    nc.vector.tensor_tensor(b[:, 0:w], cur[:, 0:w],
                                            cur[:, 0:w], OP.mult)
                    t = smalls.tile([128, 2 * NKC], F32, name=f"nt{it}")
                    nc.vector.scalar_tensor_tensor(
                        t[:, 0:w], b[:, 0:w], 1.0, hv[:, 0:w],
                        OP.mult, OP.mult)
                    nxt = r_t if it == 1 else smalls.tile(
                        [128, 2 * NKC], F32, name=f"nr{it}")
                    nc.vector.scalar_tensor_tensor(
                        nxt[:, 0:w], t[:, 0:w], 1.5, cur[:, 0:w],
                        OP.add, OP.mult)
                    cur = nxt

            batched_r(ssqQ, rQ, NQC)
            batched_r(ssqKV, rKV, 2 * NKC)
            nc.vector.tensor_scalar_mul(rkc, rKV[:, 0:NKC], SCALE_C)

            # ---- Section CS: 4 attention slabs; slab 0 also carries the
            #      q transpose work and the v scaling (they hide under PE) ----
            def q_finish(pq_tr, c):
                # scale by r_q, transpose, evict into the per-slab qT tiles
                tsrc = ln_sb.tile([128, D], BF16, name="tsrc")
                nc.vector.tensor_scalar(tsrc, cpreQ[c], rQ[:, c:c + 1],
                                        None, OP.mult)
                tpb = pq_tr.tile([128, 2 * 128], BF16, name="tpb")
                nc.tensor.transpose(tpb[:, 0:128], tsrc[:, 0:128], idn_s)
                nc.tensor.transpose(tpb[0:64, 128:256], tsrc[:, 128:192], idn_s)
                s, j = c // 4, c % 4
                jsl = slice(j * 128, (j + 1) * 128)
                if general_gb:
                    nc.vector.tensor_scalar(
                        qT0s[s][:, jsl], tpb[:, 0:128], gca, bca,
                        OP.mult, OP.add)
                    nc.vector.tensor_scalar(
                        qT1s[s][0:64, jsl], tpb[0:64, 128:256], gcb, bcb,
                        OP.mult, OP.add)
                else:
                    nc.vector.tensor_copy(qT0s[s][:, jsl], tpb[:, 0:128])
                    nc.vector.tensor_copy(qT1s[s][0:64, jsl],
                                          tpb[0:64, 128:256])

            def k_finish(pq_tr, c):
                # general-gamma/beta path only: k via transpose like q
                tsrc = ln_sb.tile([128, D], BF16, name="tsrc")
                nc.vector.tensor_scalar(tsrc, cpreK[c], rKV[:, c:c + 1],
                                        None, OP.mult)
                tpb = pq_tr.tile([128, 2 * 128], BF16, name="tpb")
                nc.tensor.transpose(tpb[:, 0:128], tsrc[:, 0:128], idn_s)
                nc.tensor.transpose(tpb[0:64, 128:256], tsrc[:, 128:192], idn_s)
                t, j = c // 4, c % 4
                jsl = slice(j * 128, (j + 1) * 128)
                nc.vector.tensor_scalar(
                    kT0t[t][:, jsl], tpb[:, 0:128], gca, bca, OP.mult, OP.add)
                nc.vector.tensor_scalar(
                    kT1t[t][0:64, jsl], tpb[0:64, 128:256], gcb, bcb,
                    OP.mult, OP.add)

            def v_finish(c):
                rj = rKV[:, NKC + c:NKC + c + 1]
                if general_gb:
                    t1 = ln_sb.tile([128, D], F32, name="t1")
                    nc.vector.tensor_scalar(t1, cpreV[c], rj, None, OP.mult)
                    t2 = ln_sb.tile([128, D], F32, name="t2")
                    nc.vector.tensor_tensor(t2, t1, gbc_s, OP.mult)
                    nc.vector.tensor_tensor(vatc[c][:, 0:192], t2, bbc_s,
                                            OP.add)
                else:
                    nc.vector.tensor_scalar(vatc[c][:, 0:192], cpreV[c], rj,
                                            None, OP.mult)

            with tc.tile_pool(name="pcs_tr", bufs=2, space="PSUM") as pcs_tr, \
                 tc.tile_pool(name="pcs_sc", bufs=2, space="PSUM") as pcs_sc, \
                 tc.tile_pool(name="pcs_oa", bufs=2, space="PSUM") as pcs_oa, \
                 tc.tile_pool(name="pcs_ob", bufs=2, space="PSUM") as pcs_ob:
                # prefix: everything slab 0's first iterations depend on
                for c in range(4):
                    q_finish(pcs_tr, c)
                if general_gb:
                    k_finish(pcs_tr, 0)
                v_finish(0)

                for s in range(NSL):
                    qsl = slice(s * SLAB, (s + 1) * SLAB)
                    oA = pcs_oa.tile([128, SLAB], F32, name="oA")
                    oB = pcs_ob.tile([65, SLAB], F32, name="oB")
                    pt_prev = None
                    for c in range(NKC):
                        if s == 0:
                            # interleave remaining LN-finish work
                            if c + 4 < NQC:
                                q_finish(pcs_tr, c + 4)
                            if general_gb and c + 1 < NKC:
                                k_finish(pcs_tr, c + 1)
                            if c + 1 < NKC:
                                v_finish(c + 1)
                        t, j = c // 4, c % 4
                        jsl = slice(j * 128, (j + 1) * 128)
                        sct = pcs_sc.tile([128, SLAB], F32, name="sct")
                        nc.tensor.matmul(sct, lhsT=kT0t[t][:, jsl],
                                         rhs=qT0s[s], start=True, stop=False)
                        nc.tensor.matmul(sct, lhsT=kT1t[t][:, jsl],
                                         rhs=qT1s[s], start=False, stop=True)
                        pt = pt_pool.tile([128, SLAB], BF16, name="pt")
                        sc_arg = SCALE_C if general_gb else rkc[:, c:c + 1]
                        nc.scalar.activation(pt, sct, FT.Exp, scale=sc_arg)
                        if pt_prev is not None:
                            cp = c - 1
                            nc.tensor.matmul(oA, lhsT=vatc[cp][:, 0:128],
                                             rhs=pt_prev, start=(cp == 0),
                                             stop=False)
                            nc.tensor.matmul(oB, lhsT=vatc[cp][:, 128:193],
                                             rhs=pt_prev, start=(cp == 0),
                                             stop=False)
                        pt_prev = pt
                    nc.tensor.matmul(oA, lhsT=vatc[NKC - 1][:, 0:128],
                                     rhs=pt_prev, start=False, stop=True)
                    nc.tensor.matmul(oB, lhsT=vatc[NKC - 1][:, 128:193],
                                     rhs=pt_prev, start=False, stop=True)
                    ea = ev.tile([128, SLAB], F32, name="ea")
                    nc.vector.tensor_copy(ea, oA)
                    eb = ev.tile([65, SLAB], F32, name="eb")
                    nc.vector.tensor_copy(eb, oB)
                    nc.sync.dma_start(outa[:, qsl], ea)
                    nc.sync.dma_start(outb[:, qsl], eb)

    nc.compile()
    return nc



def _get_program(general_gb: bool):
    key = bool(general_gb)
    if key not in _PROGRAM_CACHE:
        _PROGRAM_CACHE[key] = _build_program(key)
    return _PROGRAM_CACHE[key]


def _patchify(x):
    # (1, C, IMG, IMG) -> (S, D); token s=(i,j), feature d=(c, wi, wj)
    t = x.reshape(C, NS, WS, NS, WS)
    t = np.transpose(t, (1, 3, 0, 2, 4))
    return np.ascontiguousarray(t.reshape(S, D))


def _unpatchify(tokens):
    # (S, D) -> (1, C, IMG, IMG)
    t = tokens.reshape(NS, NS, C, WS, WS)
    t = np.transpose(t, (2, 0, 3, 1, 4))
    return np.ascontiguousarray(t.reshape(1, C, IMG, IMG))


def _prepare(inputs):
    x = np.asarray(inputs["x"], dtype=np.float32)
    Wq = np.asarray(inputs["Wq"], dtype=np.float32)
    Wk = np.asarray(inputs["Wk"], dtype=np.float32)
    Wv = np.asarray(inputs["Wv"], dtype=np.float32)
    bq = np.asarray(inputs["bq"], dtype=np.float32)
    bk = np.asarray(inputs["bk"], dtype=np.float32)
    bv = np.asarray(inputs["bv"], dtype=np.float32)
    gamma = np.asarray(inputs["gamma"], dtype=np.float32)
    beta = np.asarray(inputs["beta"], dtype=np.float32)

    general_gb = not (np.all(gamma == 1.0) and np.all(beta == 0.0))
    nc = _get_program(general_gb)

    bf = ml_dtypes.bfloat16
    xs = _patchify(x)

    # center the projection outputs by folding the per-column mean into
    # the weights: q_centered = x @ (W - colmean W)^T + (b - mean b)
    def centered(W, b):
        Wc = W - W.mean(axis=0, keepdims=True)
        bc = b - b.mean()
        return Wc, bc

    Wqc, bqc = centered(Wq, bq)
    Wkc, bkc = centered(Wk, bk)
    Wvc, bvc = centered(Wv, bv)

    # weight tensors: wa/wb = [WqT | WkT | WvT] split over the contraction
    # dim (192 -> 128 + 64), with the bias as an appended ones-row product
    wa = np.concatenate([Wqc.T[0:128], Wkc.T[0:128], Wvc.T[0:128]], axis=1)
    wb = np.zeros((128, 3 * D), np.float32)
    wb[0:64, 0:D] = Wqc.T[128:192]
    wb[0:64, D:2 * D] = Wkc.T[128:192]
    wb[0:64, 2 * D:3 * D] = Wvc.T[128:192]
    wb[64, 0:D] = bqc
    wb[64, D:2 * D] = bkc
    wb[64, 2 * D:3 * D] = bvc
    wa = wa.astype(bf)
    wb = wb.astype(bf)
    # transposed-aug k weights for the direct kT projection
    wka = Wkc.T[0:128].astype(bf)
    wkb = np.zeros((128, D), np.float32)
    wkb[0:64] = Wkc.T[128:192]
    wkb[64] = bkc
    wkb = wkb.astype(bf)
    idn = np.eye(128, dtype=bf)

    in_maps = []
    for core in range(NCORES):
        g, h = core // 2, core % 2
        seg = xs[g * SEG:(g + 1) * SEG]
        perm = np.concatenate(
            [seg[h * NQ:(h + 1) * NQ], seg[(1 - h) * NQ:(2 - h) * NQ]], axis=0)
        xsT = perm.T  # (192, 4096)
        xav = np.ascontiguousarray(xsT[0:128]).astype(bf)
        xbv = np.zeros((128, SEG), np.float32)
        xbv[0:64] = xsT[128:192]
        xbv[64] = 1.0
        xbv = xbv.astype(bf)
        im = {"xa": xav, "xb": xbv, "wa": wa, "wb": wb,
              "wka": wka, "wkb": wkb, "idn": idn}
        if general_gb:
            im["gcol"] = gamma.reshape(D, 1).copy()
            im["bcol"] = beta.reshape(D, 1).copy()
            im["gbc"] = np.broadcast_to(gamma, (128, D)).copy()
            im["bbc"] = np.broadcast_to(beta, (128, D)).copy()
        in_maps.append(im)

    return nc, in_maps, general_gb


def _postprocess(res):
    out_tokens = np.empty((S, D), np.float32)
    for core in range(NCORES):
        g, h = core // 2, core % 2
        outa = res.results[core]["outa"]  # (128, NQ) unnormalized outT
        outb = res.results[core]["outb"]  # (65, NQ): rows 0:64 outT, row 64 sums
        o_t = np.concatenate([outa, outb[0:64]], axis=0)  # (192, NQ)
        sums = outb[64]
        out_tokens[g * SEG + h * NQ: g * SEG + (h + 1) * NQ] = (o_t / sums).T

    return _unpatchify(out_tokens)


def kernel(**inputs):
    nc, in_maps, _ = _prepare(inputs)
    res = run_bass_kernel_spmd(nc, in_maps, list(range(NCORES)))
    return _postprocess(res)



# revision 10
# speedup vs baseline: 1.0031x; 1.0031x over previous
"""Trainium2 Bass kernel for a LongNet attention block.

Problem: x (1,48,256,256) -> patchify to 16384 tokens of dim 192 ->
4 segments of 4096 tokens -> q/k/v proj + LayerNorm each -> full
attention within each segment -> un-patchify.

Sharding: 2 cores per segment (8 cores, 4 segments). Each core computes
attention for 2048 queries against its segment's full 4096 keys/values.
Softmax is key-order invariant, so the host permutes each core's token
columns so its query half is always columns 0:2048 -> one SPMD program.

Fast path (gamma=1, beta=0), fp16 operands + fp8 DoubleRow out-stage:
  1. Natural q/kv projections (fp16 in, f32 PSUM) with bias via a
     ones-row; mean-centering folded into the weights on the host.
     LN sum-of-squares stats accumulate on DVE/ScalarE per chunk.
  2. Direct transposed k projection (kT). k-hat is PRE-scaled by
     C*rsqrt(ssq_k/D+eps): the r_k column vector is PE-transposed to a
     row, broadcast to [128,*] tiles via 0-stride DMA, and applied in
     the PSUM->SBUF eviction multiply. The exp then needs only a
     CONSTANT scale, enabling [128,1024] two-chunk exps.
  3. q is scaled in natural layout (per-partition tensor_scalar) and
     PE-transposed; the 64-row half is duplicated into partitions
     64:128 so the scores' second matmuls of a chunk pair can run as
     two concurrent K=64 row-tiles (array packing).
  4. Out-stage in fp8e4 DoubleRow: chunk pairs (256 keys) per matmul.
     p-hat = fp8(p * 2^psi[key]) via the exp bias AP; v-hat =
     fp8(v * r_v * 2^-psi); the v ones-column holds fp8-exact 2^-psi.
     Products are exactly compensated; the rotating quantization phase
     decorrelates RNE rounding across keys in diffuse-softmax rows.
  5. outT accumulates in PSUM; ones-column gives the denominator row.
     Host divides and transposes.
"""

import contextlib

import numpy as np
import ml_dtypes

import concourse.bacc as bacc
import concourse.mybir as mybir
import concourse.tile as tile
from concourse.bass_utils import run_bass_kernel_spmd

WS = 2
C = 48
IMG = 256
NS = IMG // WS          # 128
D = C * WS * WS         # 192
S = NS * NS             # 16384
SEG = 4096
G = S // SEG            # 4 segments
NQ = SEG // 2           # 2048 queries per core
NCORES = 8
EPS = 1e-5
SCALE_C = float(D) ** -0.5
SLAB = 512
NKC = SEG // 128        # 32 key chunks
NQC = NQ // 128         # 16 query chunks
NPAIR = NKC // 2        # 16 key-chunk pairs
NSL = NQ // SLAB        # 4 query slabs
NT = NKC // 4           # 8 key s-tiles of 512
VW = 208                # fp8 v pair tile inner width (193 used, 16B align)
SHIFT = 0.7             # exp bias shift: p = exp(s - SHIFT + psi*ln2)

F32 = mybir.dt.float32
F16 = mybir.dt.float16
BF16 = mybir.dt.bfloat16
FP8 = mybir.dt.float8e4
FT = mybir.ActivationFunctionType
OP = mybir.AluOpType
DRM = mybir.MatmulPerfMode.DoubleRow

# fp8-exact dither weights w = 2^-psi (3-mantissa-bit exact values)
_WPH = np.array([1.0, 0.9375, 0.875, 0.8125, 0.75], np.float32)
_WCOL = _WPH[np.arange(128) % 5]                      # per-partition w

_PROGRAM_CACHE = {}


def _build_program_fast():
    nc = bacc.Bacc(
        "TRN2",
        target_bir_lowering=False,
        debug=False,
        enable_asserts=False,
    )
    xa = nc.dram_tensor("xa", [128, SEG], F16, kind="ExternalInput").ap()
    xb = nc.dram_tensor("xb", [128, SEG], F16, kind="ExternalInput").ap()
    wa = nc.dram_tensor("wa", [128, 3 * D], F16, kind="ExternalInput").ap()
    wb = nc.dram_tensor("wb", [128, 3 * D], F16, kind="ExternalInput").ap()
    wka = nc.dram_tensor("wka", [128, D], F16, kind="ExternalInput").ap()
    wkb = nc.dram_tensor("wkb", [128, D], F16, kind="ExternalInput").ap()
    idn = nc.dram_tensor("idn", [128, 128], F16, kind="ExternalInput").ap()
    dithb = nc.dram_tensor("dithb", [128, 1], F32, kind="ExternalInput").ap()
    dithw = nc.dram_tensor("dithw", [128, 1], F32, kind="ExternalInput").ap()
    ones8 = nc.dram_tensor("ones8", [128, 1], FP8, kind="ExternalInput").ap()
    outa = nc.dram_tensor("outa", [128, NQ], F32, kind="ExternalOutput").ap()
    outb = nc.dram_tensor("outb", [65, NQ], F32, kind="ExternalOutput").ap()
    # DRAM scratch for the r_k row (partition-broadcast DMAs need a
    # DRAM source: SBUF APs cannot have 0-step partition dims)
    rks = nc.dram_tensor("rks", [32, 128], F16, kind="Internal").ap()

    with tile.TileContext(nc) as tc:
        with contextlib.ExitStack() as stk:
            const = stk.enter_context(tc.tile_pool(name="const", bufs=1))
            persist = stk.enter_context(tc.tile_pool(name="persist", bufs=1))
            ln_sb = stk.enter_context(tc.tile_pool(name="ln_sb", bufs=4))
            smalls = stk.enter_context(tc.tile_pool(name="smalls", bufs=4))
            pt_pool = stk.enter_context(tc.tile_pool(name="pt_pool", bufs=3))
            ev = stk.enter_context(tc.tile_pool(name="ev", bufs=4))

            xat = [const.tile([128, 512], F16, name=f"xat{t}")
                   for t in range(NT)]
            xbt = [const.tile([128, 512], F16, name=f"xbt{t}")
                   for t in range(NT)]
            for t in range(NT):
                tsl = slice(t * 512, (t + 1) * 512)
                nc.sync.dma_start(xat[t], xa[:, tsl])
                nc.sync.dma_start(xbt[t], xb[:, tsl])
            wa_s = const.tile([128, 3 * D], F16)
            nc.sync.dma_start(wa_s, wa)
            wb_s = const.tile([128, 3 * D], F16)
            nc.sync.dma_start(wb_s, wb)
            wka_s = const.tile([128, D], F16)
            nc.sync.dma_start(wka_s, wka)
            wkb_s = const.tile([128, D], F16)
            nc.sync.dma_start(wkb_s, wkb)
            idn_s = const.tile([128, 128], F16)
            nc.sync.dma_start(idn_s, idn)
            dithb_s = const.tile([128, 1], F32)
            nc.sync.dma_start(dithb_s, dithb)
            dithw_s = const.tile([128, 1], F32)
            nc.sync.dma_start(dithw_s, dithw)
            epsc = const.tile([128, 1], F32)
            nc.gpsimd.memset(epsc, EPS)
            halfc = const.tile([128, 1], F32)
            nc.gpsimd.memset(halfc, 0.5)

            # persistent state
            qT0s = [persist.tile([128, SLAB], F16, name=f"qT0s{s}")
                    for s in range(NSL)]
            qT1s = [persist.tile([128, SLAB], F16, name=f"qT1s{s}")
                    for s in range(NSL)]
            kT0t = [persist.tile([128, 512], F16, name=f"kT0t{t}")
                    for t in range(NT)]
            # pair layout: block b in {0,1} holds chunks (4t+2b, 4t+2b+1)
            # at rows 0:64 / 64:128 (k-hat dims 128:192)
            kT1t = [persist.tile([128, 256], F16, name=f"kT1t{t}")
                    for t in range(NT)]
            # unscaled kT staging (scaled once r_k broadcasts are ready)
            kTu0 = [persist.tile([128, 512], F16, name=f"kTu0{t}")
                    for t in range(NT)]
            kTu1 = [persist.tile([128, 256], F16, name=f"kTu1{t}")
                    for t in range(NT)]
            # fp8 v pair tiles: [key-in-chunk, which-chunk, 192 dims+ones]
            vat8 = [persist.tile([128, 2, VW], FP8, name=f"vat8{j}")
                    for j in range(NPAIR)]
            cpreQ = [persist.tile([128, D], F16, name=f"cpreQ{c}")
                     for c in range(NQC)]
            cpreV = [persist.tile([128, D], F16, name=f"cpreV{c}")
                     for c in range(NKC)]
            kbc = [persist.tile([128, 512], F16, name=f"kbc{t}")
                   for t in range(NT)]
            rowk = persist.tile([32, 128], F16)
            ssqQ = persist.tile([128, NQC], F32)
            ssqKV = persist.tile([128, 2 * NKC], F32)  # k: 0:32, v: 32:64
            rQ = persist.tile([128, NQC], F32)
            rKV = persist.tile([128, 2 * NKC], F32)
            rkc16 = persist.tile([128, NKC], F16)   # f16(C / std_k) columns
            rvd = persist.tile([128, NKC], F32)     # r_v * 2^-psi columns
            for j in range(NPAIR):
                nc.sync.dma_start(vat8[j][:, 0, 192:193], ones8)
                nc.sync.dma_start(vat8[j][:, 1, 192:193], ones8)

            # ---- Section A: projections + sum-of-squares stats ----
            def kt_proj(pool, t):
                kp0 = pool.tile([128, 512], F32, name="kp0")
                nc.tensor.matmul(kp0, lhsT=wka_s[:, 0:128], rhs=xat[t],
                                 start=True, stop=False)
                nc.tensor.matmul(kp0, lhsT=wkb_s[:, 0:128], rhs=xbt[t],
                                 start=False, stop=True)
                # dims 128:192 per 128-key chunk at alternating partition
                # halves so the pair tiles evict without partition shifts
                kp1 = pool.tile([128, 512], F32, name="kp1")
                for j in range(4):
                    r0 = (j % 2) * 64
                    csl = slice(j * 128, (j + 1) * 128)
                    nc.tensor.matmul(kp1[r0:r0 + 64, csl],
                                     lhsT=wka_s[:, 128:192],
                                     rhs=xat[t][:, csl],
                                     start=True, stop=False)
                    nc.tensor.matmul(kp1[r0:r0 + 64, csl],
                                     lhsT=wkb_s[:, 128:192],
                                     rhs=xbt[t][:, csl],
                                     start=False, stop=True)
                # evict unscaled (scale applied after r_k is known)
                nc.vector.tensor_copy(kTu0[t], kp0)
                for j in range(4):
                    r0 = (j % 2) * 64
                    rs = slice(r0, r0 + 64)
                    pb = j // 2
                    nc.vector.tensor_copy(
                        kTu1[t][rs, pb * 128:(pb + 1) * 128],
                        kp1[rs, j * 128:(j + 1) * 128])

            with tc.tile_pool(name="pa_raw", bufs=3, space="PSUM") as pa_raw, \
                 tc.tile_pool(name="pa_kt", bufs=1, space="PSUM") as pa_kt:
                for c in range(NQC):
                    jsl = slice((c % 4) * 128, (c % 4 + 1) * 128)
                    raw = pa_raw.tile([128, D], F32, name="rawQ")
                    nc.tensor.matmul(raw, lhsT=xat[c // 4][:, jsl],
                                     rhs=wa_s[:, 0:D], start=True, stop=False)
                    nc.tensor.matmul(raw, lhsT=xbt[c // 4][:, jsl],
                                     rhs=wb_s[:, 0:D], start=False, stop=True)
                    nc.vector.tensor_copy(cpreQ[c], raw)
                    # q stats from the fp16 eviction (cheap 2x DVE mode)
                    sqd = ln_sb.tile([128, D], F16, name="sqd")
                    nc.vector.scalar_tensor_tensor(
                        sqd, cpreQ[c], 1.0, cpreQ[c], OP.mult, OP.mult,
                        accum_out=ssqQ[:, c:c + 1])
                for c in range(NKC):
                    jsl = slice((c % 4) * 128, (c % 4 + 1) * 128)
                    raw = pa_raw.tile([128, 2 * D], F32, name="rawKV")
                    nc.tensor.matmul(raw, lhsT=xat[c // 4][:, jsl],
                                     rhs=wa_s[:, D:3 * D],
                                     start=True, stop=False)
                    nc.tensor.matmul(raw, lhsT=xbt[c // 4][:, jsl],
                                     rhs=wb_s[:, D:3 * D],
                                     start=False, stop=True)
                    # k stats: ScalarE square from PSUM on even chunks,
                    # DVE (SBUF-staged) on odd — balances the engines
                    if c % 2 == 0:
                        sqd = ln_sb.tile([128, D], F16, name="sqd")
                        nc.scalar.activation(sqd, raw[:, 0:D], FT.Square,
                                             accum_out=ssqKV[:, c:c + 1])
                    else:
                        kst = ln_sb.tile([128, D], F16, name="kst")
                        nc.vector.tensor_copy(kst, raw[:, 0:D])
                        sqd = ln_sb.tile([128, D], F16, name="sqd")
                        nc.vector.scalar_tensor_tensor(
                            sqd, kst, 1.0, kst, OP.mult, OP.mult,
                            accum_out=ssqKV[:, c:c + 1])
                    nc.vector.tensor_copy(cpreV[c], raw[:, D:2 * D])
                    sqd2 = ln_sb.tile([128, D], F16, name="sqd2")
                    nc.vector.scalar_tensor_tensor(
                        sqd2, cpreV[c], 1.0, cpreV[c], OP.mult, OP.mult,
                        accum_out=ssqKV[:, NKC + c:NKC + c + 1])
                    if c % 4 == 3:
                        kt_proj(pa_kt, c // 4)

            # ---- batched rsqrt: r = rsqrt(ssq/D + eps) ----
            if True:
                def batched_r(ssq_t, r_t, w):
                    vv = smalls.tile([128, 2 * NKC], F32, name="vv")
                    nc.vector.tensor_scalar(vv[:, 0:w], ssq_t[:, 0:w],
                                            1.0 / D, EPS, OP.mult, OP.add)
                    nc.scalar.activation(r_t[:, 0:w], vv[:, 0:w], FT.Exp,
                                         scale=-0.5, bias=halfc)
                    hv = smalls.tile([128, 2 * NKC], F32, name="hv")
                    nc.vector.tensor_scalar(hv[:, 0:w], vv[:, 0:w], -0.5,
                                            None, OP.mult)
                    cur = r_t
                    for it in range(2):
                        b = smalls.tile([128, 2 * NKC], F32, name=f"nb{it}")
                        nc.vector.tensor_tensor(b[:, 0:w], cur[:, 0:w],
                                                cur[:, 0:w], OP.mult)
                        t2 = smalls.tile([128, 2 * NKC], F32, name=f"nt{it}")
                        nc.vector.scalar_tensor_tensor(
                            t2[:, 0:w], b[:, 0:w], 1.0, hv[:, 0:w],
                            OP.mult, OP.mult)
                        nxt = r_t if it == 1 else smalls.tile(
                            [128, 2 * NKC], F32, name=f"nr{it}")
                        nc.vector.scalar_tensor_tensor(
                            nxt[:, 0:w], t2[:, 0:w], 1.5, cur[:, 0:w],
                            OP.add, OP.mult)
                        cur = nxt

                batched_r(ssqQ, rQ, NQC)
                batched_r(ssqKV, rKV, 2 * NKC)
                # k-hat scale columns (f16) and dithered v scale columns
                nc.vector.tensor_scalar(rkc16, rKV[:, 0:NKC], SCALE_C,
                                        None, OP.mult)
                nc.vector.tensor_scalar(rvd, rKV[:, NKC:2 * NKC], dithw_s,
                                        None, OP.mult)

                # r_k columns -> row chunks -> broadcast tiles
                with tc.tile_pool(name="pa_rt", bufs=1,
                                  space="PSUM") as pa_rt:
                    trp = pa_rt.tile([32, 128], F16, name="trp")
                    nc.tensor.transpose(trp, rkc16, idn_s)
                    nc.vector.tensor_copy(rowk, trp)
                nc.sync.dma_start(rks, rowk)
                for t in range(NT):
                    for j in range(4):
                        r = 4 * t + j
                        nc.sync.dma_start(
                            kbc[t][:, j * 128:(j + 1) * 128],
                            rks[r:r + 1, :].to_broadcast((128, 128)))

                # k-hat rescale pass (SBUF->SBUF fp16, 2x DVE mode)
                for t in range(NT):
                    nc.vector.tensor_tensor(kT0t[t], kTu0[t], kbc[t],
                                            OP.mult)
                    for pb in range(2):
                        csl = slice(pb * 128, (pb + 1) * 128)
                        # rows 0:64 <- even chunk (2pb), 64:128 <- odd
                        nc.vector.tensor_tensor(
                            kT1t[t][0:64, csl], kTu1[t][0:64, csl],
                            kbc[t][0:64, (2 * pb) * 128:(2 * pb + 1) * 128],
                            OP.mult)
                        nc.vector.tensor_tensor(
                            kT1t[t][64:128, csl], kTu1[t][64:128, csl],
                            kbc[t][64:128,
                                   (2 * pb + 1) * 128:(2 * pb + 2) * 128],
                            OP.mult)

            # ---- v finish: dithered fp8 evictions into pair tiles ----
            for c in range(NKC):
                nc.vector.tensor_scalar(
                    vat8[c // 2][:, c % 2, 0:D], cpreV[c],
                    rvd[:, c:c + 1], None, OP.mult)

            # ---- q finish: scale, transpose, evict (dup 64-row half) ----
            with tc.tile_pool(name="pq_tr", bufs=4, space="PSUM") as pq_tr:
                for c in range(NQC):
                    tsrc = ln_sb.tile([128, D], F16, name="tsrc")
                    nc.vector.tensor_scalar(tsrc, cpreQ[c], rQ[:, c:c + 1],
                                            None, OP.mult)
                    tpb = pq_tr.tile([128, 2 * 128], F16, name="tpb")
                    nc.tensor.transpose(tpb[:, 0:128], tsrc[:, 0:128], idn_s)
                    nc.tensor.transpose(tpb[0:64, 128:256],
                                        tsrc[:, 128:192], idn_s)
                    s, j = c // 4, c % 4
                    jsl = slice(j * 128, (j + 1) * 128)
                    nc.vector.tensor_copy(qT0s[s][:, jsl], tpb[:, 0:128])
                    nc.vector.tensor_copy(qT1s[s][0:64, jsl],
                                          tpb[0:64, 128:256])
                    nc.vector.tensor_copy(qT1s[s][64:128, jsl],
                                          tpb[0:64, 128:256])

            # ---- attention: 4 slabs x 16 chunk pairs ----
            with tc.tile_pool(name="pcs_sc", bufs=2, space="PSUM") as pcs_sc, \
                 tc.tile_pool(name="pcs_oa", bufs=2, space="PSUM") as pcs_oa, \
                 tc.tile_pool(name="pcs_ob", bufs=2, space="PSUM") as pcs_ob:
                for s in range(NSL):
                    qsl = slice(s * SLAB, (s + 1) * SLAB)
                    oA = pcs_oa.tile([128, SLAB], F32, name="oA")
                    oB = pcs_ob.tile([65, SLAB], F32, name="oB")
                    for pr in range(NPAIR):
                        c0 = 2 * pr
                        t = c0 // 4
                        pb = (c0 % 4) // 2   # pair block within kT1t[t]
                        sct = pcs_sc.tile([128, 1024], F32, name="sct")
                        for h in range(2):
                            c = c0 + h
                            jsl = slice((c % 4) * 128, (c % 4 + 1) * 128)
                            ssl = slice(h * 512, (h + 1) * 512)
                            nc.tensor.matmul(sct[:, ssl],
                                             lhsT=kT0t[t][:, jsl],
                                             rhs=qT0s[s],
                                             start=True, stop=False)
                        # second halves: two concurrent K=64 row tiles
                        psl = slice(pb * 128, (pb + 1) * 128)
                        nc.tensor.matmul(sct[:, 0:512],
                                         lhsT=kT1t[t][0:64, psl],
                                         rhs=qT1s[s][0:64, :],
                                         start=False, stop=True)
                        nc.tensor.matmul(sct[:, 512:1024],
                                         lhsT=kT1t[t][64:128, psl],
                                         rhs=qT1s[s][64:128, :],
                                         start=False, stop=True)
                        pt = pt_pool.tile([128, 1024], FP8, name="pt")
                        nc.scalar.activation(pt, sct, FT.Exp, bias=dithb_s)
                        pt3 = pt.rearrange("p (a b) -> p a b", a=2)
                        nc.tensor.matmul(oA, lhsT=vat8[pr][:, :, 0:128],
                                         rhs=pt3, start=(pr == 0),
                                         stop=(pr == NPAIR - 1),
                                         perf_mode=DRM)
                        nc.tensor.matmul(oB, lhsT=vat8[pr][:, :, 128:193],
                                         rhs=pt3, start=(pr == 0),
                                         stop=(pr == NPAIR - 1),
                                         perf_mode=DRM)
                    ea = ev.tile([128, SLAB], F32, name="ea")
                    nc.vector.tensor_copy(ea, oA)
                    eb = ev.tile([65, SLAB], F32, name="eb")
                    nc.vector.tensor_copy(eb, oB)
                    nc.sync.dma_start(outa[:, qsl], ea)
                    nc.sync.dma_start(outb[:, qsl], eb)

    nc.compile()
    return nc


def _build_program_legacy(general_gb: bool):
    """Baseline bf16 kernel (handles general gamma/beta)."""
    nc = bacc.Bacc(
        "TRN2",
        target_bir_lowering=False,
        debug=False,
        enable_asserts=False,
    )
    xa = nc.dram_tensor("xa", [128, SEG], BF16, kind="ExternalInput").ap()
    xb = nc.dram_tensor("xb", [128, SEG], BF16, kind="ExternalInput").ap()
    wa = nc.dram_tensor("wa", [128, 3 * D], BF16, kind="ExternalInput").ap()
    wb = nc.dram_tensor("wb", [128, 3 * D], BF16, kind="ExternalInput").ap()
    wka = nc.dram_tensor("wka", [128, D], BF16, kind="ExternalInput").ap()
    wkb = nc.dram_tensor("wkb", [128, D], BF16, kind="ExternalInput").ap()
    idn = nc.dram_tensor("idn", [128, 128], BF16, kind="ExternalInput").ap()
    if general_gb:
        gcol = nc.dram_tensor("gcol", [D, 1], F32, kind="ExternalInput").ap()
        bcol = nc.dram_tensor("bcol", [D, 1], F32, kind="ExternalInput").ap()
        gbc = nc.dram_tensor("gbc", [128, D], F32, kind="ExternalInput").ap()
        bbc = nc.dram_tensor("bbc", [128, D], F32, kind="ExternalInput").ap()
    outa = nc.dram_tensor("outa", [128, NQ], F32, kind="ExternalOutput").ap()
    outb = nc.dram_tensor("outb", [65, NQ], F32, kind="ExternalOutput").ap()

    NSLl = NQ // SLAB
    VWl = 200

    with tile.TileContext(nc) as tc:
        with contextlib.ExitStack() as stk:
            const = stk.enter_context(tc.tile_pool(name="const", bufs=1))
            persist = stk.enter_context(tc.tile_pool(name="persist", bufs=1))
            ln_sb = stk.enter_context(tc.tile_pool(name="ln_sb", bufs=4))
            smalls = stk.enter_context(tc.tile_pool(name="smalls", bufs=4))
            pt_pool = stk.enter_context(tc.tile_pool(name="pt_pool", bufs=4))
            ev = stk.enter_context(tc.tile_pool(name="ev", bufs=4))

            xat = [const.tile([128, 512], BF16, name=f"xat{t}")
                   for t in range(NKC // 4)]
            xbt = [const.tile([128, 512], BF16, name=f"xbt{t}")
                   for t in range(NKC // 4)]
            for t in range(NKC // 4):
                tsl = slice(t * 512, (t + 1) * 512)
                nc.sync.dma_start(xat[t], xa[:, tsl])
                nc.sync.dma_start(xbt[t], xb[:, tsl])
            wa_s = const.tile([128, 3 * D], BF16)
            nc.sync.dma_start(wa_s, wa)
            wb_s = const.tile([128, 3 * D], BF16)
            nc.sync.dma_start(wb_s, wb)
            wka_s = const.tile([128, D], BF16)
            nc.sync.dma_start(wka_s, wka)
            wkb_s = const.tile([128, D], BF16)
            nc.sync.dma_start(wkb_s, wkb)
            idn_s = const.tile([128, 128], BF16)
            nc.sync.dma_start(idn_s, idn)
            epsc = const.tile([128, 1], F32)
            nc.gpsimd.memset(epsc, EPS)
            halfc = const.tile([128, 1], F32)
            nc.gpsimd.memset(halfc, 0.5)
            if general_gb:
                gca = const.tile([128, 1], F32)
                nc.sync.dma_start(gca, gcol[0:128])
                gcb = const.tile([64, 1], F32)
                nc.sync.dma_start(gcb, gcol[128:192])
                bca = const.tile([128, 1], F32)
                nc.sync.dma_start(bca, bcol[0:128])
                bcb = const.tile([64, 1], F32)
                nc.sync.dma_start(bcb, bcol[128:192])
                gbc_s = const.tile([128, D], F32)
                nc.sync.dma_start(gbc_s, gbc)
                bbc_s = const.tile([128, D], F32)
                nc.sync.dma_start(bbc_s, bbc)

            qT0s = [persist.tile([128, SLAB], BF16, name=f"qT0s{s}")
                    for s in range(NSLl)]
            qT1s = [persist.tile([128, SLAB], BF16, name=f"qT1s{s}")
                    for s in range(NSLl)]
            kT0t = [persist.tile([128, 512], BF16, name=f"kT0t{t}")
                    for t in range(NT)]
            kT1t = [persist.tile([128, 512], BF16, name=f"kT1t{t}")
                    for t in range(NT)]
            vatc = [persist.tile([128, VWl], BF16, name=f"vatc{c}")
                    for c in range(NKC)]
            cpreQ = [persist.tile([128, D], BF16, name=f"cpreQ{c}")
                     for c in range(NQC)]
            cpreV = [persist.tile([128, D], BF16, name=f"cpreV{c}")
                     for c in range(NKC)]
            cpreK = ([persist.tile([128, D], BF16, name=f"cpreK{c}")
                      for c in range(NKC)] if general_gb else None)
            ssqQ = persist.tile([128, NQC], F32)
            ssqKV = persist.tile([128, 2 * NKC], F32)
            rQ = persist.tile([128, NQC], F32)
            rKV = persist.tile([128, 2 * NKC], F32)
            rkc = persist.tile([128, NKC], F32)
            for s in range(NSLl):
                nc.gpsimd.memset(qT1s[s][64:128, :], 0.0)
            for t in range(NT):
                nc.gpsimd.memset(kT1t[t][64:128, :], 0.0)
            for c in range(NKC):
                nc.gpsimd.memset(vatc[c][:, 192:193], 1.0)

            def kt_proj(pool, t):
                kp0 = pool.tile([128, 512], F32, name="kp0")
                nc.tensor.matmul(kp0, lhsT=wka_s[:, 0:128], rhs=xat[t],
                                 start=True, stop=False)
                nc.tensor.matmul(kp0, lhsT=wkb_s[:, 0:128], rhs=xbt[t],
                                 start=False, stop=True)
                kp1 = pool.tile([64, 512], F32, name="kp1")
                nc.tensor.matmul(kp1, lhsT=wka_s[:, 128:192], rhs=xat[t],
                                 start=True, stop=False)
                nc.tensor.matmul(kp1, lhsT=wkb_s[:, 128:192], rhs=xbt[t],
                                 start=False, stop=True)
                nc.vector.tensor_copy(kT0t[t], kp0)
                nc.vector.tensor_copy(kT1t[t][0:64, :], kp1)

            with tc.tile_pool(name="pa_raw", bufs=3, space="PSUM") as pa_raw, \
                 tc.tile_pool(name="pa_kt", bufs=1, space="PSUM") as pa_kt:
                for c in range(NQC):
                    jsl = slice((c % 4) * 128, (c % 4 + 1) * 128)
                    raw = pa_raw.tile([128, D], F32, name="rawQ")
                    nc.tensor.matmul(raw, lhsT=xat[c // 4][:, jsl],
                                     rhs=wa_s[:, 0:D], start=True, stop=False)
                    nc.tensor.matmul(raw, lhsT=xbt[c // 4][:, jsl],
                                     rhs=wb_s[:, 0:D], start=False, stop=True)
                    nc.vector.tensor_copy(cpreQ[c], raw)
                    sqd = ln_sb.tile([128, D], BF16, name="sqd")
                    nc.scalar.activation(sqd, raw, FT.Square,
                                         accum_out=ssqQ[:, c:c + 1])
                    if not general_gb and c % 2 == 1:
                        kt_proj(pa_kt, c // 2)
                for c in range(NKC):
                    jsl = slice((c % 4) * 128, (c % 4 + 1) * 128)
                    raw = pa_raw.tile([128, 2 * D], F32, name="rawKV")
                    nc.tensor.matmul(raw, lhsT=xat[c // 4][:, jsl],
                                     rhs=wa_s[:, D:3 * D],
                                     start=True, stop=False)
                    nc.tensor.matmul(raw, lhsT=xbt[c // 4][:, jsl],
                                     rhs=wb_s[:, D:3 * D],
                                     start=False, stop=True)
                    sqd = ln_sb.tile([128, D], BF16, name="sqd")
                    nc.scalar.activation(sqd, raw[:, 0:D], FT.Square,
                                         accum_out=ssqKV[:, c:c + 1])
                    if general_gb:
                        nc.vector.tensor_copy(cpreK[c], raw[:, 0:D])
                    nc.vector.tensor_copy(cpreV[c], raw[:, D:2 * D])
                    if c % 2 == 0:
                        sqd2 = ln_sb.tile([128, D], BF16, name="sqd2")
                        nc.vector.scalar_tensor_tensor(
                            sqd2, cpreV[c], 1.0, cpreV[c], OP.mult, OP.mult,
                            accum_out=ssqKV[:, NKC + c:NKC + c + 1])
                    else:
                        sqd2 = ln_sb.tile([128, D], BF16, name="sqd2")
                        nc.scalar.activation(
                            sqd2, raw[:, D:2 * D], FT.Square,
                            accum_out=ssqKV[:, NKC + c:NKC + c + 1])

            def batched_r(ssq_t, r_t, w):
                vv = smalls.tile([128, 2 * NKC], F32, name="vv")
                nc.vector.tensor_scalar(vv[:, 0:w], ssq_t[:, 0:w], 1.0 / D,
                                        EPS, OP.mult, OP.add)
                nc.scalar.activation(r_t[:, 0:w], vv[:, 0:w], FT.Exp,
                                     scale=-0.5, bias=halfc)
                hv = smalls.tile([128, 2 * NKC], F32, name="hv")
                nc.vector.tensor_scalar(hv[:, 0:w], vv[:, 0:w], -0.5, None,
                                        OP.mult)
                cur = r_t
                for it in range(2):
                    b = smalls.tile([128, 2 * NKC], F32, name=f"nb{it}")
                    nc.vector.tensor_tensor(b[:, 0:w], cur[:, 0:w],
                                            cur[:, 0:w], OP.mult)
                    t = smalls.tile([128, 2 * NKC], F32, name=f"nt{it}")
                    nc.vector.scalar_tensor_tensor(
                        t[:, 0:w], b[:, 0:w], 1.0, hv[:, 0:w],
                        OP.mult, OP.mult)
                    nxt = r_t if it == 1 else smalls.tile(
                        [128, 2 * NKC], F32, name=f"nr{it}")
                    nc.vector.scalar_tensor_tensor(
                        nxt[:, 0:w], t[:, 0:w], 1.5, cur[:, 0:w],
                        OP.add, OP.mult)
                    cur = nxt

            batched_r(ssqQ, rQ, NQC)
            batched_r(ssqKV, rKV, 2 * NKC)
            nc.vector.tensor_scalar_mul(rkc, rKV[:, 0:NKC], SCALE_C)

            def q_finish(pq_tr, c):
                tsrc = ln_sb.tile([128, D], BF16, name="tsrc")
                nc.vector.tensor_scalar(tsrc, cpreQ[c], rQ[:, c:c + 1],
                                        None, OP.mult)
                tpb = pq_tr.tile([128, 2 * 128], BF16, name="tpb")
                nc.tensor.transpose(tpb[:, 0:128], tsrc[:, 0:128], idn_s)
                nc.tensor.transpose(tpb[0:64, 128:256], tsrc[:, 128:192],
                                    idn_s)
                s, j = c // 4, c % 4
                jsl = slice(j * 128, (j + 1) * 128)
                if general_gb:
                    nc.vector.tensor_scalar(
                        qT0s[s][:, jsl], tpb[:, 0:128], gca, bca,
                        OP.mult, OP.add)
                    nc.vector.tensor_scalar(
                        qT1s[s][0:64, jsl], tpb[0:64, 128:256], gcb, bcb,
                        OP.mult, OP.add)
                else:
                    nc.vector.tensor_copy(qT0s[s][:, jsl], tpb[:, 0:128])
                    nc.vector.tensor_copy(qT1s[s][0:64, jsl],
                                          tpb[0:64, 128:256])

            def k_finish(pq_tr, c):
                tsrc = ln_sb.tile([128, D], BF16, name="tsrc")
                nc.vector.tensor_scalar(tsrc, cpreK[c], rKV[:, c:c + 1],
                                        None, OP.mult)
                tpb = pq_tr.tile([128, 2 * 128], BF16, name="tpb")
                nc.tensor.transpose(tpb[:, 0:128], tsrc[:, 0:128], idn_s)
                nc.tensor.transpose(tpb[0:64, 128:256], tsrc[:, 128:192],
                                    idn_s)
                t, j = c // 4, c % 4
                jsl = slice(j * 128, (j + 1) * 128)
                nc.vector.tensor_scalar(
                    kT0t[t][:, jsl], tpb[:, 0:128], gca, bca,
                    OP.mult, OP.add)
                nc.vector.tensor_scalar(
                    kT1t[t][0:64, jsl], tpb[0:64, 128:256], gcb, bcb,
                    OP.mult, OP.add)

            def v_finish(c):
                rj = rKV[:, NKC + c:NKC + c + 1]
                if general_gb:
                    t1 = ln_sb.tile([128, D], F32, name="t1")
                    nc.vector.tensor_scalar(t1, cpreV[c], rj, None, OP.mult)
                    t2 = ln_sb.tile([128, D], F32, name="t2")
                    nc.vector.tensor_tensor(t2, t1, gbc_s, OP.mult)
                    nc.vector.tensor_tensor(vatc[c][:, 0:192], t2, bbc_s,
                                            OP.add)
                else:
                    nc.vector.tensor_scalar(vatc[c][:, 0:192], cpreV[c], rj,
                                            None, OP.mult)

            with tc.tile_pool(name="pcs_tr", bufs=2, space="PSUM") as pcs_tr, \
                 tc.tile_pool(name="pcs_sc", bufs=2, space="PSUM") as pcs_sc, \
                 tc.tile_pool(name="pcs_oa", bufs=2, space="PSUM") as pcs_oa, \
                 tc.tile_pool(name="pcs_ob", bufs=2, space="PSUM") as pcs_ob:
                for c in range(4):
                    q_finish(pcs_tr, c)
                if general_gb:
                    k_finish(pcs_tr, 0)
                v_finish(0)

                for s in range(NSLl):
                    qsl = slice(s * SLAB, (s + 1) * SLAB)
                    oA = pcs_oa.tile([128, SLAB], F32, name="oA")
                    oB = pcs_ob.tile([65, SLAB], F32, name="oB")
                    pt_prev = None
                    for c in range(NKC):
                        if s == 0:
                            if c + 4 < NQC:
                                q_finish(pcs_tr, c + 4)
                            if general_gb and c + 1 < NKC:
                                k_finish(pcs_tr, c + 1)
                            if c + 1 < NKC:
                                v_finish(c + 1)
                        t, j = c // 4, c % 4
                        jsl = slice(j * 128, (j + 1) * 128)
                        sct = pcs_sc.tile([128, SLAB], F32, name="sct")
                        nc.tensor.matmul(sct, lhsT=kT0t[t][:, jsl],
                                         rhs=qT0s[s], start=True, stop=False)
                        nc.tensor.matmul(sct, lhsT=kT1t[t][:, jsl],
                                         rhs=qT1s[s], start=False, stop=True)
                        pt = pt_pool.tile([128, SLAB], BF16, name="pt")
                        sc_arg = SCALE_C if general_gb else rkc[:, c:c + 1]
                        nc.scalar.activation(pt, sct, FT.Exp, scale=sc_arg)
                        if pt_prev is not None:
                            cp = c - 1
                            nc.tensor.matmul(oA, lhsT=vatc[cp][:, 0:128],
                                             rhs=pt_prev, start=(cp == 0),
                                             stop=False)
                            nc.tensor.matmul(oB, lhsT=vatc[cp][:, 128:193],
                                             rhs=pt_prev, start=(cp == 0),
                                             stop=False)
                        pt_prev = pt
                    nc.tensor.matmul(oA, lhsT=vatc[NKC - 1][:, 0:128],
                                     rhs=pt_prev, start=False, stop=True)
                    nc.tensor.matmul(oB, lhsT=vatc[NKC - 1][:, 128:193],
                                     rhs=pt_prev, start=False, stop=True)
                    ea = ev.tile([128, SLAB], F32, name="ea")
                    nc.vector.tensor_copy(ea, oA)
                    eb = ev.tile([65, SLAB], F32, name="eb")
                    nc.vector.tensor_copy(eb, oB)
                    nc.sync.dma_start(outa[:, qsl], ea)
                    nc.sync.dma_start(outb[:, qsl], eb)

    nc.compile()
    return nc


def _get_program(general_gb: bool):
    key = bool(general_gb)
    if key not in _PROGRAM_CACHE:
        if key:
            _PROGRAM_CACHE[key] = _build_program_legacy(True)
        else:
            _PROGRAM_CACHE[key] = _build_program_fast()
    return _PROGRAM_CACHE[key]


def _patchify(x):
    # (1, C, IMG, IMG) -> (S, D); token s=(i,j), feature d=(c, wi, wj)
    t = x.reshape(C, NS, WS, NS, WS)
    t = np.transpose(t, (1, 3, 0, 2, 4))
    return np.ascontiguousarray(t.reshape(S, D))


def _unpatchify(tokens):
    # (S, D) -> (1, C, IMG, IMG)
    t = tokens.reshape(NS, NS, C, WS, WS)
    t = np.transpose(t, (2, 0, 3, 1, 4))
    return np.ascontiguousarray(t.reshape(1, C, IMG, IMG))


def _prepare(inputs):
    x = np.asarray(inputs["x"], dtype=np.float32)
    Wq = np.asarray(inputs["Wq"], dtype=np.float32)
    Wk = np.asarray(inputs["Wk"], dtype=np.float32)
    Wv = np.asarray(inputs["Wv"], dtype=np.float32)
    bq = np.asarray(inputs["bq"], dtype=np.float32)
    bk = np.asarray(inputs["bk"], dtype=np.float32)
    bv = np.asarray(inputs["bv"], dtype=np.float32)
    gamma = np.asarray(inputs["gamma"], dtype=np.float32)
    beta = np.asarray(inputs["beta"], dtype=np.float32)

    general_gb = not (np.all(gamma == 1.0) and np.all(beta == 0.0))
    nc = _get_program(general_gb)

    dt = np.float16 if not general_gb else ml_dtypes.bfloat16
    xs = _patchify(x)

    def centered(W, b):
        Wc = W - W.mean(axis=0, keepdims=True)
        bc = b - b.mean()
        return Wc, bc

    Wqc, bqc = centered(Wq, bq)
    Wkc, bkc = centered(Wk, bk)
    Wvc, bvc = centered(Wv, bv)

    wa = np.concatenate([Wqc.T[0:128], Wkc.T[0:128], Wvc.T[0:128]], axis=1)
    wb = np.zeros((128, 3 * D), np.float32)
    wb[0:64, 0:D] = Wqc.T[128:192]
    wb[0:64, D:2 * D] = Wkc.T[128:192]
    wb[0:64, 2 * D:3 * D] = Wvc.T[128:192]
    wb[64, 0:D] = bqc
    wb[64, D:2 * D] = bkc
    wb[64, 2 * D:3 * D] = bvc
    wa = wa.astype(dt)
    wb = wb.astype(dt)
    wka = Wkc.T[0:128].astype(dt)
    wkb = np.zeros((128, D), np.float32)
    wkb[0:64] = Wkc.T[128:192]
    wkb[64] = bkc
    wkb = wkb.astype(dt)
    idn = np.eye(128, dtype=dt)

    dithb = (-SHIFT - np.log(_WCOL)).reshape(128, 1).astype(np.float32)
    dithw = _WCOL.reshape(128, 1).astype(np.float32)
    ones8 = _WCOL.reshape(128, 1).astype(ml_dtypes.float8_e4m3)

    in_maps = []
    for core in range(NCORES):
        g, h = core // 2, core % 2
        seg = xs[g * SEG:(g + 1) * SEG]
        perm = np.concatenate(
            [seg[h * NQ:(h + 1) * NQ], seg[(1 - h) * NQ:(2 - h) * NQ]],
            axis=0)
        xsT = perm.T  # (192, 4096)
        xav = np.ascontiguousarray(xsT[0:128]).astype(dt)
        xbv = np.zeros((128, SEG), np.float32)
        xbv[0:64] = xsT[128:192]
        xbv[64] = 1.0
        xbv = xbv.astype(dt)
        im = {"xa": xav, "xb": xbv, "wa": wa, "wb": wb,
              "wka": wka, "wkb": wkb, "idn": idn}
        if general_gb:
            im["gcol"] = gamma.reshape(D, 1).copy()
            im["bcol"] = beta.reshape(D, 1).copy()
            im["gbc"] = np.broadcast_to(gamma, (128, D)).copy()
            im["bbc"] = np.broadcast_to(beta, (128, D)).copy()
        else:
            im["dithb"] = dithb
            im["dithw"] = dithw
            im["ones8"] = ones8
        in_maps.append(im)

    return nc, in_maps, general_gb


def _postprocess(res):
    out_tokens = np.empty((S, D), np.float32)
    for core in range(NCORES):
        g, h = core // 2, core % 2
        outa = res.results[core]["outa"]  # (128, NQ) unnormalized outT
        outb = res.results[core]["outb"]  # (65, NQ): 0:64 outT, row 64 sums
        o_t = np.concatenate([outa, outb[0:64]], axis=0)  # (192, NQ)
        sums = outb[64]
        out_tokens[g * SEG + h * NQ: g * SEG + (h + 1) * NQ] = \
            (o_t / sums).T

    return _unpatchify(out_tokens)


def kernel(**inputs):
    nc, in_maps, _ = _prepare(inputs)
    res = run_bass_kernel_spmd(nc, in_maps, list(range(NCORES)))
    return _postprocess(res)


# revision 15
# speedup vs baseline: 1.1099x; 1.1065x over previous
"""Trainium2 Bass kernel for a LongNet attention block.

Problem: x (1,48,256,256) -> patchify to 16384 tokens of dim 192 ->
4 segments of 4096 tokens -> q/k/v proj + LayerNorm each -> full
attention within each segment -> un-patchify.

Sharding: 2 cores per segment (8 cores, 4 segments). Each core computes
attention for 2048 queries against its segment's full 4096 keys/values.
Softmax is key-order invariant, so the host permutes each core's token
columns so its query half is always columns 0:2048 -> one SPMD program.

Fast path (gamma=1, beta=0), fp16 operands + fp8 DoubleRow out-stage:
  1. Natural q/kv projections (fp16 in, f32 PSUM) with bias via a
     ones-row; mean-centering folded into the weights on the host.
     LN sum-of-squares stats accumulate on DVE/ScalarE per chunk.
  2. Direct transposed k projection (kT). k-hat is PRE-scaled by
     C*rsqrt(ssq_k/D+eps): the r_k column vector is PE-transposed to a
     row, broadcast to [128,*] tiles via 0-stride DMA, and applied in
     the PSUM->SBUF eviction multiply. The exp then needs only a
     CONSTANT scale, enabling [128,1024] two-chunk exps.
  3. q is scaled in natural layout (per-partition tensor_scalar) and
     PE-transposed; the 64-row half is duplicated into partitions
     64:128 so the scores' second matmuls of a chunk pair can run as
     two concurrent K=64 row-tiles (array packing).
  4. Out-stage in fp8e4 DoubleRow: chunk pairs (256 keys) per matmul.
     p-hat = fp8(p * 2^psi[key]) via the exp bias AP; v-hat =
     fp8(v * r_v * 2^-psi); the v ones-column holds fp8-exact 2^-psi.
     Products are exactly compensated; the rotating quantization phase
     decorrelates RNE rounding across keys in diffuse-softmax rows.
  5. outT accumulates in PSUM; ones-column gives the denominator row.
     Host divides and transposes.
"""

import contextlib

import numpy as np
import ml_dtypes

import concourse.bacc as bacc
import concourse.mybir as mybir
import concourse.tile as tile
from concourse.bass_utils import run_bass_kernel_spmd

WS = 2
C = 48
IMG = 256
NS = IMG // WS          # 128
D = C * WS * WS         # 192
S = NS * NS             # 16384
SEG = 4096
G = S // SEG            # 4 segments
NQ = SEG // 2           # 2048 queries per core
NCORES = 8
EPS = 1e-5
SCALE_C = float(D) ** -0.5
SLAB = 512
NKC = SEG // 128        # 32 key chunks
NQC = NQ // 128         # 16 query chunks
NPAIR = NKC // 2        # 16 key-chunk pairs
NSL = NQ // SLAB        # 4 query slabs
NT = NKC // 4           # 8 key s-tiles of 512
VW = 208                # fp8 v pair tile inner width (193 used, 16B align)
SHIFT = 0.7             # exp bias shift: p = exp(s - SHIFT + psi*ln2)

F32 = mybir.dt.float32
F16 = mybir.dt.float16
BF16 = mybir.dt.bfloat16
FP8 = mybir.dt.float8e4
FT = mybir.ActivationFunctionType
OP = mybir.AluOpType
DRM = mybir.MatmulPerfMode.DoubleRow

# fp8-exact dither weights w = 2^-psi (3-mantissa-bit exact values)
_WPH = np.array([1.0, 0.9375, 0.875, 0.8125, 0.75], np.float32)
_WCOL = _WPH[np.arange(128) % 5]                      # per-partition w

_PROGRAM_CACHE = {}


def _build_program_fast():
    nc = bacc.Bacc(
        "TRN2",
        target_bir_lowering=False,
        debug=False,
        enable_asserts=False,
    )
    xa = nc.dram_tensor("xa", [128, SEG], F16, kind="ExternalInput").ap()
    xb = nc.dram_tensor("xb", [128, SEG], F16, kind="ExternalInput").ap()
    wa = nc.dram_tensor("wa", [128, 3 * D], F16, kind="ExternalInput").ap()
    wb = nc.dram_tensor("wb", [128, 3 * D], F16, kind="ExternalInput").ap()
    wka = nc.dram_tensor("wka", [128, D], F16, kind="ExternalInput").ap()
    wkb = nc.dram_tensor("wkb", [128, D], F16, kind="ExternalInput").ap()
    idn = nc.dram_tensor("idn", [128, 128], F16, kind="ExternalInput").ap()
    dithb = nc.dram_tensor("dithb", [128, 1], F32, kind="ExternalInput").ap()
    dithw = nc.dram_tensor("dithw", [128, 1], F32, kind="ExternalInput").ap()
    ones8 = nc.dram_tensor("ones8", [128, 1], FP8, kind="ExternalInput").ap()
    outa = nc.dram_tensor("outa", [128, NQ], F32, kind="ExternalOutput").ap()
    outb = nc.dram_tensor("outb", [65, NQ], F32, kind="ExternalOutput").ap()
    # DRAM scratch for the r_k row (partition-broadcast DMAs need a
    # DRAM source: SBUF APs cannot have 0-step partition dims)
    rks = nc.dram_tensor("rks", [32, 128], F16, kind="Internal").ap()

    with tile.TileContext(nc) as tc:
        with contextlib.ExitStack() as stk:
            const = stk.enter_context(tc.tile_pool(name="const", bufs=1))
            persist = stk.enter_context(tc.tile_pool(name="persist", bufs=1))
            ln_sb = stk.enter_context(tc.tile_pool(name="ln_sb", bufs=4))
            smalls = stk.enter_context(tc.tile_pool(name="smalls", bufs=4))
            pt_pool = stk.enter_context(tc.tile_pool(name="pt_pool", bufs=4))
            ev = stk.enter_context(tc.tile_pool(name="ev", bufs=4))

            # weights first so the first projection matmuls can start
            # as soon as the first token tile lands
            wa_s = const.tile([128, 3 * D], F16)
            nc.sync.dma_start(wa_s, wa)
            wb_s = const.tile([128, 3 * D], F16)
            nc.sync.dma_start(wb_s, wb)
            wka_s = const.tile([128, D], F16)
            nc.sync.dma_start(wka_s, wka)
            wkb_s = const.tile([128, D], F16)
            nc.sync.dma_start(wkb_s, wkb)
            idn_s = const.tile([128, 128], F16)
            nc.sync.dma_start(idn_s, idn)
            xat = [const.tile([128, 512], F16, name=f"xat{t}")
                   for t in range(NT)]
            xbt = [const.tile([128, 512], F16, name=f"xbt{t}")
                   for t in range(NT)]
            for t in range(NT):
                tsl = slice(t * 512, (t + 1) * 512)
                nc.sync.dma_start(xat[t], xa[:, tsl])
                nc.sync.dma_start(xbt[t], xb[:, tsl])
            dithb_s = const.tile([128, 1], F32)
            nc.sync.dma_start(dithb_s, dithb)
            dithw_s = const.tile([128, 1], F32)
            nc.sync.dma_start(dithw_s, dithw)
            epsc = const.tile([128, 1], F32)
            nc.gpsimd.memset(epsc, EPS)
            halfc = const.tile([128, 1], F32)
            nc.gpsimd.memset(halfc, 0.5)

            # persistent state
            qT0s = [persist.tile([128, SLAB], F16, name=f"qT0s{s}")
                    for s in range(NSL)]
            qT1s = [persist.tile([128, SLAB], F16, name=f"qT1s{s}")
                    for s in range(NSL)]
            kT0t = [persist.tile([128, 512], F16, name=f"kT0t{t}")
                    for t in range(NT)]
            # pair layout: block b in {0,1} holds chunks (4t+2b, 4t+2b+1)
            # at rows 0:64 / 64:128 (k-hat dims 128:192)
            kT1t = [persist.tile([128, 256], F16, name=f"kT1t{t}")
                    for t in range(NT)]

            # fp8 v pair tiles: [key-in-chunk, which-chunk, 192 dims+ones]
            vat8 = [persist.tile([128, 2, VW], FP8, name=f"vat8{j}")
                    for j in range(NPAIR)]
            cpreQ = [persist.tile([128, D], F16, name=f"cpreQ{c}")
                     for c in range(NQC)]
            cpreV = [persist.tile([128, D], F16, name=f"cpreV{c}")
                     for c in range(NKC)]
            kbc = [persist.tile([128, 512], F16, name=f"kbc{t}")
                   for t in range(NT)]
            rowk = persist.tile([32, 128], F16)
            ssqQ = persist.tile([128, NQC], F32)
            ssqKV = persist.tile([128, 2 * NKC], F32)  # k: 0:32, v: 32:64
            rQ = persist.tile([128, NQC], F32)
            rKV = persist.tile([128, 2 * NKC], F32)
            rkc16 = persist.tile([128, NKC], F16)   # f16(C / std_k) columns
            rvd = persist.tile([128, NKC], F32)     # r_v * 2^-psi columns
            for j in range(NPAIR):
                nc.sync.dma_start(vat8[j][:, 0, 192:193], ones8)
                nc.sync.dma_start(vat8[j][:, 1, 192:193], ones8)

            # ---- batched rsqrt: r = rsqrt(ssq/D + eps) ----
            def batched_r(ssq_t, r_t, w):
                vv = smalls.tile([128, 2 * NKC], F32, name="vv")
                nc.vector.tensor_scalar(vv[:, 0:w], ssq_t[:, 0:w],
                                        1.0 / D, EPS, OP.mult, OP.add)
                nc.scalar.activation(r_t[:, 0:w], vv[:, 0:w], FT.Exp,
                                     scale=-0.5, bias=halfc)
                hv = smalls.tile([128, 2 * NKC], F32, name="hv")
                nc.vector.tensor_scalar(hv[:, 0:w], vv[:, 0:w], -0.5,
                                        None, OP.mult)
                cur = r_t
                for it in range(2):
                    b = smalls.tile([128, 2 * NKC], F32, name=f"nb{it}")
                    nc.vector.tensor_tensor(b[:, 0:w], cur[:, 0:w],
                                            cur[:, 0:w], OP.mult)
                    t2 = smalls.tile([128, 2 * NKC], F32, name=f"nt{it}")
                    nc.vector.scalar_tensor_tensor(
                        t2[:, 0:w], b[:, 0:w], 1.0, hv[:, 0:w],
                        OP.mult, OP.mult)
                    nxt = r_t if it == 1 else smalls.tile(
                        [128, 2 * NKC], F32, name=f"nr{it}")
                    nc.vector.scalar_tensor_tensor(
                        nxt[:, 0:w], t2[:, 0:w], 1.5, cur[:, 0:w],
                        OP.add, OP.mult)
                    cur = nxt

            # ---- Section A: projections + stats + finishes ----
            with tc.tile_pool(name="pa_raw", bufs=3, space="PSUM") as pa_raw, \
                 tc.tile_pool(name="pq_tr", bufs=2, space="PSUM") as pq_tr:
                for c in range(NQC):
                    jsl = slice((c % 4) * 128, (c % 4 + 1) * 128)
                    raw = pa_raw.tile([128, D], F32, name="rawQ")
                    nc.tensor.matmul(raw, lhsT=xat[c // 4][:, jsl],
                                     rhs=wa_s[:, 0:D], start=True, stop=False)
                    nc.tensor.matmul(raw, lhsT=xbt[c // 4][:, jsl],
                                     rhs=wb_s[:, 0:D], start=False, stop=True)
                    nc.vector.tensor_copy(cpreQ[c], raw)
                    # q stats from the fp16 eviction (cheap 2x DVE mode)
                    sqd = ln_sb.tile([128, D], F16, name="sqd")
                    nc.vector.scalar_tensor_tensor(
                        sqd, cpreQ[c], 1.0, cpreQ[c], OP.mult, OP.mult,
                        accum_out=ssqQ[:, c:c + 1])
                # early q Newton so q finishes overlap the kv projections
                batched_r(ssqQ, rQ, NQC)

                for c in range(NKC):
                    jsl = slice((c % 4) * 128, (c % 4 + 1) * 128)
                    raw = pa_raw.tile([128, 2 * D], F32, name="rawKV")
                    nc.tensor.matmul(raw, lhsT=xat[c // 4][:, jsl],
                                     rhs=wa_s[:, D:3 * D],
                                     start=True, stop=False)
                    nc.tensor.matmul(raw, lhsT=xbt[c // 4][:, jsl],
                                     rhs=wb_s[:, D:3 * D],
                                     start=False, stop=True)
                    # k stats on ScalarE (idle during section A)
                    sqd = ln_sb.tile([128, D], F16, name="sqd")
                    nc.scalar.activation(sqd, raw[:, 0:D], FT.Square,
                                         accum_out=ssqKV[:, c:c + 1])
                    nc.vector.tensor_copy(cpreV[c], raw[:, D:2 * D])
                    sqd2 = ln_sb.tile([128, D], F16, name="sqd2")
                    nc.vector.scalar_tensor_tensor(
                        sqd2, cpreV[c], 1.0, cpreV[c], OP.mult, OP.mult,
                        accum_out=ssqKV[:, NKC + c:NKC + c + 1])
                    # interleave q finishes (need only rQ)
                    if c % 2 == 0:
                        cq = c // 2
                        tsrc = ln_sb.tile([128, D], F16, name="tsrc")
                        nc.vector.tensor_scalar(tsrc, cpreQ[cq],
                                                rQ[:, cq:cq + 1],
                                                None, OP.mult)
                        tpb = pq_tr.tile([128, 2 * 128], F16, name="tpb")
                        nc.tensor.transpose(tpb[:, 0:128], tsrc[:, 0:128],
                                            idn_s)
                        nc.tensor.transpose(tpb[0:64, 128:256],
                                            tsrc[:, 128:192], idn_s)
                        s, j = cq // 4, cq % 4
                        jsl = slice(j * 128, (j + 1) * 128)
                        nc.vector.tensor_copy(qT0s[s][:, jsl],
                                              tpb[:, 0:128])
                        nc.scalar.copy(qT1s[s][0:64, jsl],
                                       tpb[0:64, 128:256])
                        nc.scalar.copy(qT1s[s][64:128, jsl],
                                       tpb[0:64, 128:256])

                batched_r(ssqKV, rKV, 2 * NKC)
                # k-hat scale columns (f16) and dithered v scale columns
                nc.vector.tensor_scalar(rkc16, rKV[:, 0:NKC], SCALE_C,
                                        None, OP.mult)
                nc.vector.tensor_scalar(rvd, rKV[:, NKC:2 * NKC], dithw_s,
                                        None, OP.mult)
                # remaining q finishes
                for cq in range(NKC // 2, NQC):
                    tsrc = ln_sb.tile([128, D], F16, name="tsrc")
                    nc.vector.tensor_scalar(tsrc, cpreQ[cq],
                                            rQ[:, cq:cq + 1], None, OP.mult)
                    tpb = pq_tr.tile([128, 2 * 128], F16, name="tpb")
                    nc.tensor.transpose(tpb[:, 0:128], tsrc[:, 0:128], idn_s)
                    nc.tensor.transpose(tpb[0:64, 128:256],
                                        tsrc[:, 128:192], idn_s)
                    s, j = cq // 4, cq % 4
                    jsl = slice(j * 128, (j + 1) * 128)
                    nc.vector.tensor_copy(qT0s[s][:, jsl], tpb[:, 0:128])
                    nc.scalar.copy(qT1s[s][0:64, jsl], tpb[0:64, 128:256])
                    nc.scalar.copy(qT1s[s][64:128, jsl], tpb[0:64, 128:256])

            # ---- v finish on ScalarE: copy with per-partition scale ----
            for c in range(NKC):
                nc.scalar.activation(vat8[c // 2][:, c % 2, 0:D], cpreV[c],
                                     FT.Copy, scale=rvd[:, c:c + 1])

            # ---- kT projection with direct scaled eviction ----
            with tc.tile_pool(name="pa_kt", bufs=2, space="PSUM") as pa_kt, \
                 tc.tile_pool(name="pa_rt", bufs=1, space="PSUM") as pa_rt:
                trp = pa_rt.tile([32, 128], F16, name="trp")
                nc.tensor.transpose(trp, rkc16, idn_s)
                nc.vector.tensor_copy(rowk, trp)
                nc.sync.dma_start(rks, rowk)
                for t in range(NT):
                    for j in range(4):
                        r = 4 * t + j
                        nc.sync.dma_start(
                            kbc[t][:, j * 128:(j + 1) * 128],
                            rks[r:r + 1, :].to_broadcast((128, 128)))
                for t in range(NT):
                    kp0 = pa_kt.tile([128, 512], F32, name="kp0")
                    nc.tensor.matmul(kp0, lhsT=wka_s[:, 0:128], rhs=xat[t],
                                     start=True, stop=False)
                    nc.tensor.matmul(kp0, lhsT=wkb_s[:, 0:128], rhs=xbt[t],
                                     start=False, stop=True)
                    kp1 = pa_kt.tile([128, 512], F32, name="kp1")
                    for j in range(4):
                        r0 = (j % 2) * 64
                        csl = slice(j * 128, (j + 1) * 128)
                        nc.tensor.matmul(kp1[r0:r0 + 64, csl],
                                         lhsT=wka_s[:, 128:192],
                                         rhs=xat[t][:, csl],
                                         start=True, stop=False)
                        nc.tensor.matmul(kp1[r0:r0 + 64, csl],
                                         lhsT=wkb_s[:, 128:192],
                                         rhs=xbt[t][:, csl],
                                         start=False, stop=True)
                    nc.vector.tensor_tensor(kT0t[t], kp0, kbc[t], OP.mult)
                    for j in range(4):
                        r0 = (j % 2) * 64
                        rs = slice(r0, r0 + 64)
                        pb = j // 2
                        nc.vector.tensor_tensor(
                            kT1t[t][rs, pb * 128:(pb + 1) * 128],
                            kp1[rs, j * 128:(j + 1) * 128],
                            kbc[t][rs, j * 128:(j + 1) * 128],
                            OP.mult)

            # ---- attention: 4 slabs x 16 chunk pairs ----
            with tc.tile_pool(name="pcs_sc", bufs=3, space="PSUM") as pcs_sc, \
                 tc.tile_pool(name="pcs_oa", bufs=1, space="PSUM") as pcs_oa, \
                 tc.tile_pool(name="pcs_ob", bufs=1, space="PSUM") as pcs_ob:
                for s in range(NSL):
                    qsl = slice(s * SLAB, (s + 1) * SLAB)
                    oA = pcs_oa.tile([128, SLAB], F32, name="oA")
                    oB = pcs_ob.tile([65, SLAB], F32, name="oB")
                    for pr in range(NPAIR):
                        c0 = 2 * pr
                        t = c0 // 4
                        pb = (c0 % 4) // 2   # pair block within kT1t[t]
                        sct = pcs_sc.tile([128, 1024], F32, name="sct")
                        for h in range(2):
                            c = c0 + h
                            jsl = slice((c % 4) * 128, (c % 4 + 1) * 128)
                            ssl = slice(h * 512, (h + 1) * 512)
                            nc.tensor.matmul(sct[:, ssl],
                                             lhsT=kT0t[t][:, jsl],
                                             rhs=qT0s[s],
                                             start=True, stop=False)
                        # second halves: two concurrent K=64 row tiles
                        psl = slice(pb * 128, (pb + 1) * 128)
                        nc.tensor.matmul(sct[:, 0:512],
                                         lhsT=kT1t[t][0:64, psl],
                                         rhs=qT1s[s][0:64, :],
                                         start=False, stop=True)
                        nc.tensor.matmul(sct[:, 512:1024],
                                         lhsT=kT1t[t][64:128, psl],
                                         rhs=qT1s[s][64:128, :],
                                         start=False, stop=True)
                        pt = pt_pool.tile([128, 1024], FP8, name="pt")
                        nc.scalar.activation(pt, sct, FT.Exp, bias=dithb_s)
                        pt3 = pt.rearrange("p (a b) -> p a b", a=2)
                        nc.tensor.matmul(oA, lhsT=vat8[pr][:, :, 0:128],
                                         rhs=pt3, start=(pr == 0),
                                         stop=(pr == NPAIR - 1),
                                         perf_mode=DRM)
                        nc.tensor.matmul(oB, lhsT=vat8[pr][:, :, 128:193],
                                         rhs=pt3, start=(pr == 0),
                                         stop=(pr == NPAIR - 1),
                                         perf_mode=DRM)
                    ea = ev.tile([128, SLAB], F32, name="ea")
                    nc.vector.tensor_copy(ea, oA)
                    eb = ev.tile([65, SLAB], F32, name="eb")
                    nc.vector.tensor_copy(eb, oB)
                    nc.sync.dma_start(outa[:, qsl], ea)
                    nc.sync.dma_start(outb[:, qsl], eb)

    nc.compile()
    return nc


def _build_program_legacy(general_gb: bool):
    """Baseline bf16 kernel (handles general gamma/beta)."""
    nc = bacc.Bacc(
        "TRN2",
        target_bir_lowering=False,
        debug=False,
        enable_asserts=False,
    )
    xa = nc.dram_tensor("xa", [128, SEG], BF16, kind="ExternalInput").ap()
    xb = nc.dram_tensor("xb", [128, SEG], BF16, kind="ExternalInput").ap()
    wa = nc.dram_tensor("wa", [128, 3 * D], BF16, kind="ExternalInput").ap()
    wb = nc.dram_tensor("wb", [128, 3 * D], BF16, kind="ExternalInput").ap()
    wka = nc.dram_tensor("wka", [128, D], BF16, kind="ExternalInput").ap()
    wkb = nc.dram_tensor("wkb", [128, D], BF16, kind="ExternalInput").ap()
    idn = nc.dram_tensor("idn", [128, 128], BF16, kind="ExternalInput").ap()
    if general_gb:
        gcol = nc.dram_tensor("gcol", [D, 1], F32, kind="ExternalInput").ap()
        bcol = nc.dram_tensor("bcol", [D, 1], F32, kind="ExternalInput").ap()
        gbc = nc.dram_tensor("gbc", [128, D], F32, kind="ExternalInput").ap()
        bbc = nc.dram_tensor("bbc", [128, D], F32, kind="ExternalInput").ap()
    outa = nc.dram_tensor("outa", [128, NQ], F32, kind="ExternalOutput").ap()
    outb = nc.dram_tensor("outb", [65, NQ], F32, kind="ExternalOutput").ap()

    NSLl = NQ // SLAB
    VWl = 200

    with tile.TileContext(nc) as tc:
        with contextlib.ExitStack() as stk:
            const = stk.enter_context(tc.tile_pool(name="const", bufs=1))
            persist = stk.enter_context(tc.tile_pool(name="persist", bufs=1))
            ln_sb = stk.enter_context(tc.tile_pool(name="ln_sb", bufs=4))
            smalls = stk.enter_context(tc.tile_pool(name="smalls", bufs=4))
            pt_pool = stk.enter_context(tc.tile_pool(name="pt_pool", bufs=4))
            ev = stk.enter_context(tc.tile_pool(name="ev", bufs=4))

            xat = [const.tile([128, 512], BF16, name=f"xat{t}")
                   for t in range(NKC // 4)]
            xbt = [const.tile([128, 512], BF16, name=f"xbt{t}")
                   for t in range(NKC // 4)]
            for t in range(NKC // 4):
                tsl = slice(t * 512, (t + 1) * 512)
                nc.sync.dma_start(xat[t], xa[:, tsl])
                nc.sync.dma_start(xbt[t], xb[:, tsl])
            wa_s = const.tile([128, 3 * D], BF16)
            nc.sync.dma_start(wa_s, wa)
            wb_s = const.tile([128, 3 * D], BF16)
            nc.sync.dma_start(wb_s, wb)
            wka_s = const.tile([128, D], BF16)
            nc.sync.dma_start(wka_s, wka)
            wkb_s = const.tile([128, D], BF16)
            nc.sync.dma_start(wkb_s, wkb)
            idn_s = const.tile([128, 128], BF16)
            nc.sync.dma_start(idn_s, idn)
            epsc = const.tile([128, 1], F32)
            nc.gpsimd.memset(epsc, EPS)
            halfc = const.tile([128, 1], F32)
            nc.gpsimd.memset(halfc, 0.5)
            if general_gb:
                gca = const.tile([128, 1], F32)
                nc.sync.dma_start(gca, gcol[0:128])
                gcb = const.tile([64, 1], F32)
                nc.sync.dma_start(gcb, gcol[128:192])
                bca = const.tile([128, 1], F32)
                nc.sync.dma_start(bca, bcol[0:128])
                bcb = const.tile([64, 1], F32)
                nc.sync.dma_start(bcb, bcol[128:192])
                gbc_s = const.tile([128, D], F32)
                nc.sync.dma_start(gbc_s, gbc)
                bbc_s = const.tile([128, D], F32)
                nc.sync.dma_start(bbc_s, bbc)

            qT0s = [persist.tile([128, SLAB], BF16, name=f"qT0s{s}")
                    for s in range(NSLl)]
            qT1s = [persist.tile([128, SLAB], BF16, name=f"qT1s{s}")
                    for s in range(NSLl)]
            kT0t = [persist.tile([128, 512], BF16, name=f"kT0t{t}")
                    for t in range(NT)]
            kT1t = [persist.tile([128, 512], BF16, name=f"kT1t{t}")
                    for t in range(NT)]
            vatc = [persist.tile([128, VWl], BF16, name=f"vatc{c}")
                    for c in range(NKC)]
            cpreQ = [persist.tile([128, D], BF16, name=f"cpreQ{c}")
                     for c in range(NQC)]
            cpreV = [persist.tile([128, D], BF16, name=f"cpreV{c}")
                     for c in range(NKC)]
            cpreK = ([persist.tile([128, D], BF16, name=f"cpreK{c}")
                      for c in range(NKC)] if general_gb else None)
            ssqQ = persist.tile([128, NQC], F32)
            ssqKV = persist.tile([128, 2 * NKC], F32)
            rQ = persist.tile([128, NQC], F32)
            rKV = persist.tile([128, 2 * NKC], F32)
            rkc = persist.tile([128, NKC], F32)
            for s in range(NSLl):
                nc.gpsimd.memset(qT1s[s][64:128, :], 0.0)
            for t in range(NT):
                nc.gpsimd.memset(kT1t[t][64:128, :], 0.0)
            for c in range(NKC):
                nc.gpsimd.memset(vatc[c][:, 192:193], 1.0)

            def kt_proj(pool, t):
                kp0 = pool.tile([128, 512], F32, name="kp0")
                nc.tensor.matmul(kp0, lhsT=wka_s[:, 0:128], rhs=xat[t],
                                 start=True, stop=False)
                nc.tensor.matmul(kp0, lhsT=wkb_s[:, 0:128], rhs=xbt[t],
                                 start=False, stop=True)
                kp1 = pool.tile([64, 512], F32, name="kp1")
                nc.tensor.matmul(kp1, lhsT=wka_s[:, 128:192], rhs=xat[t],
                                 start=True, stop=False)
                nc.tensor.matmul(kp1, lhsT=wkb_s[:, 128:192], rhs=xbt[t],
                                 start=False, stop=True)
                nc.vector.tensor_copy(kT0t[t], kp0)
                nc.vector.tensor_copy(kT1t[t][0:64, :], kp1)

            with tc.tile_pool(name="pa_raw", bufs=3, space="PSUM") as pa_raw, \
                 tc.tile_pool(name="pa_kt", bufs=1, space="PSUM") as pa_kt:
                for c in range(NQC):
                    jsl = slice((c % 4) * 128, (c % 4 + 1) * 128)
                    raw = pa_raw.tile([128, D], F32, name="rawQ")
                    nc.tensor.matmul(raw, lhsT=xat[c // 4][:, jsl],
                                     rhs=wa_s[:, 0:D], start=True, stop=False)
                    nc.tensor.matmul(raw, lhsT=xbt[c // 4][:, jsl],
                                     rhs=wb_s[:, 0:D], start=False, stop=True)
                    nc.vector.tensor_copy(cpreQ[c], raw)
                    sqd = ln_sb.tile([128, D], BF16, name="sqd")
                    nc.scalar.activation(sqd, raw, FT.Square,
                                         accum_out=ssqQ[:, c:c + 1])
                    if not general_gb and c % 2 == 1:
                        kt_proj(pa_kt, c // 2)
                for c in range(NKC):
                    jsl = slice((c % 4) * 128, (c % 4 + 1) * 128)
                    raw = pa_raw.tile([128, 2 * D], F32, name="rawKV")
                    nc.tensor.matmul(raw, lhsT=xat[c // 4][:, jsl],
                                     rhs=wa_s[:, D:3 * D],
                                     start=True, stop=False)
                    nc.tensor.matmul(raw, lhsT=xbt[c // 4][:, jsl],
                                     rhs=wb_s[:, D:3 * D],
                                     start=False, stop=True)
                    sqd = ln_sb.tile([128, D], BF16, name="sqd")
                    nc.scalar.activation(sqd, raw[:, 0:D], FT.Square,
                                         accum_out=ssqKV[:, c:c + 1])
                    if general_gb:
                        nc.vector.tensor_copy(cpreK[c], raw[:, 0:D])
                    nc.vector.tensor_copy(cpreV[c], raw[:, D:2 * D])
                    if c % 2 == 0:
                        sqd2 = ln_sb.tile([128, D], BF16, name="sqd2")
                        nc.vector.scalar_tensor_tensor(
                            sqd2, cpreV[c], 1.0, cpreV[c], OP.mult, OP.mult,
                            accum_out=ssqKV[:, NKC + c:NKC + c + 1])
                    else:
                        sqd2 = ln_sb.tile([128, D], BF16, name="sqd2")
                        nc.scalar.activation(
                            sqd2, raw[:, D:2 * D], FT.Square,
                            accum_out=ssqKV[:, NKC + c:NKC + c + 1])

            def batched_r(ssq_t, r_t, w):
                vv = smalls.tile([128, 2 * NKC], F32, name="vv")
                nc.vector.tensor_scalar(vv[:, 0:w], ssq_t[:, 0:w], 1.0 / D,
                                        EPS, OP.mult, OP.add)
                nc.scalar.activation(r_t[:, 0:w], vv[:, 0:w], FT.Exp,
                                     scale=-0.5, bias=halfc)
                hv = smalls.tile([128, 2 * NKC], F32, name="hv")
                nc.vector.tensor_scalar(hv[:, 0:w], vv[:, 0:w], -0.5, None,
                                        OP.mult)
                cur = r_t
                for it in range(2):
                    b = smalls.tile([128, 2 * NKC], F32, name=f"nb{it}")
                    nc.vector.tensor_tensor(b[:, 0:w], cur[:, 0:w],
                                            cur[:, 0:w], OP.mult)
                    t = smalls.tile([128, 2 * NKC], F32, name=f"nt{it}")
                    nc.vector.scalar_tensor_tensor(
                        t[:, 0:w], b[:, 0:w], 1.0, hv[:, 0:w],
                        OP.mult, OP.mult)
                    nxt = r_t if it == 1 else smalls.tile(
                        [128, 2 * NKC], F32, name=f"nr{it}")
                    nc.vector.scalar_tensor_tensor(
                        nxt[:, 0:w], t[:, 0:w], 1.5, cur[:, 0:w],
                        OP.add, OP.mult)
                    cur = nxt

            batched_r(ssqQ, rQ, NQC)
            batched_r(ssqKV, rKV, 2 * NKC)
            nc.vector.tensor_scalar_mul(rkc, rKV[:, 0:NKC], SCALE_C)

            def q_finish(pq_tr, c):
                tsrc = ln_sb.tile([128, D], BF16, name="tsrc")
                nc.vector.tensor_scalar(tsrc, cpreQ[c], rQ[:, c:c + 1],
                                        None, OP.mult)
                tpb = pq_tr.tile([128, 2 * 128], BF16, name="tpb")
                nc.tensor.transpose(tpb[:, 0:128], tsrc[:, 0:128], idn_s)
                nc.tensor.transpose(tpb[0:64, 128:256], tsrc[:, 128:192],
                                    idn_s)
                s, j = c // 4, c % 4
                jsl = slice(j * 128, (j + 1) * 128)
                if general_gb:
                    nc.vector.tensor_scalar(
                        qT0s[s][:, jsl], tpb[:, 0:128], gca, bca,
                        OP.mult, OP.add)
                    nc.vector.tensor_scalar(
                        qT1s[s][0:64, jsl], tpb[0:64, 128:256], gcb, bcb,
                        OP.mult, OP.add)
                else:
                    nc.vector.tensor_copy(qT0s[s][:, jsl], tpb[:, 0:128])
                    nc.vector.tensor_copy(qT1s[s][0:64, jsl],
                                          tpb[0:64, 128:256])

            def k_finish(pq_tr, c):
                tsrc = ln_sb.tile([128, D], BF16, name="tsrc")
                nc.vector.tensor_scalar(tsrc, cpreK[c], rKV[:, c:c + 1],
                                        None, OP.mult)
                tpb = pq_tr.tile([128, 2 * 128], BF16, name="tpb")
                nc.tensor.transpose(tpb[:, 0:128], tsrc[:, 0:128], idn_s)
                nc.tensor.transpose(tpb[0:64, 128:256], tsrc[:, 128:192],
                                    idn_s)
                t, j = c // 4, c % 4
                jsl = slice(j * 128, (j + 1) * 128)
                nc.vector.tensor_scalar(
                    kT0t[t][:, jsl], tpb[:, 0:128], gca, bca,
                    OP.mult, OP.add)
                nc.vector.tensor_scalar(
                    kT1t[t][0:64, jsl], tpb[0:64, 128:256], gcb, bcb,
                    OP.mult, OP.add)

            def v_finish(c):
                rj = rKV[:, NKC + c:NKC + c + 1]
                if general_gb:
                    t1 = ln_sb.tile([128, D], F32, name="t1")
                    nc.vector.tensor_scalar(t1, cpreV[c], rj, None, OP.mult)
                    t2 = ln_sb.tile([128, D], F32, name="t2")
                    nc.vector.tensor_tensor(t2, t1, gbc_s, OP.mult)
                    nc.vector.tensor_tensor(vatc[c][:, 0:192], t2, bbc_s,
                                            OP.add)
                else:
                    nc.vector.tensor_scalar(vatc[c][:, 0:192], cpreV[c], rj,
                                            None, OP.mult)

            with tc.tile_pool(name="pcs_tr", bufs=2, space="PSUM") as pcs_tr, \
                 tc.tile_pool(name="pcs_sc", bufs=2, space="PSUM") as pcs_sc, \
                 tc.tile_pool(name="pcs_oa", bufs=2, space="PSUM") as pcs_oa, \
                 tc.tile_pool(name="pcs_ob", bufs=2, space="PSUM") as pcs_ob:
                for c in range(4):
                    q_finish(pcs_tr, c)
                if general_gb:
                    k_finish(pcs_tr, 0)
                v_finish(0)

                for s in range(NSLl):
                    qsl = slice(s * SLAB, (s + 1) * SLAB)
                    oA = pcs_oa.tile([128, SLAB], F32, name="oA")
                    oB = pcs_ob.tile([65, SLAB], F32, name="oB")
                    pt_prev = None
                    for c in range(NKC):
                        if s == 0:
                            if c + 4 < NQC:
                                q_finish(pcs_tr, c + 4)
                            if general_gb and c + 1 < NKC:
                                k_finish(pcs_tr, c + 1)
                            if c + 1 < NKC:
                                v_finish(c + 1)
                        t, j = c // 4, c % 4
                        jsl = slice(j * 128, (j + 1) * 128)
                        sct = pcs_sc.tile([128, SLAB], F32, name="sct")
                        nc.tensor.matmul(sct, lhsT=kT0t[t][:, jsl],
                                         rhs=qT0s[s], start=True, stop=False)
                        nc.tensor.matmul(sct, lhsT=kT1t[t][:, jsl],
                                         rhs=qT1s[s], start=False, stop=True)
                        pt = pt_pool.tile([128, SLAB], BF16, name="pt")
                        sc_arg = SCALE_C if general_gb else rkc[:, c:c + 1]
                        nc.scalar.activation(pt, sct, FT.Exp, scale=sc_arg)
                        if pt_prev is not None:
                            cp = c - 1
                            nc.tensor.matmul(oA, lhsT=vatc[cp][:, 0:128],
                                             rhs=pt_prev, start=(cp == 0),
                                             stop=False)
                            nc.tensor.matmul(oB, lhsT=vatc[cp][:, 128:193],
                                             rhs=pt_prev, start=(cp == 0),
                                             stop=False)
                        pt_prev = pt
                    nc.tensor.matmul(oA, lhsT=vatc[NKC - 1][:, 0:128],
                                     rhs=pt_prev, start=False, stop=True)
                    nc.tensor.matmul(oB, lhsT=vatc[NKC - 1][:, 128:193],
                                     rhs=pt_prev, start=False, stop=True)
                    ea = ev.tile([128, SLAB], F32, name="ea")
                    nc.vector.tensor_copy(ea, oA)
                    eb = ev.tile([65, SLAB], F32, name="eb")
                    nc.vector.tensor_copy(eb, oB)
                    nc.sync.dma_start(outa[:, qsl], ea)
                    nc.sync.dma_start(outb[:, qsl], eb)

    nc.compile()
    return nc


def _get_program(general_gb: bool):
    key = bool(general_gb)
    if key not in _PROGRAM_CACHE:
        if key:
            _PROGRAM_CACHE[key] = _build_program_legacy(True)
        else:
            _PROGRAM_CACHE[key] = _build_program_fast()
    return _PROGRAM_CACHE[key]


def _patchify(x):
    # (1, C, IMG, IMG) -> (S, D); token s=(i,j), feature d=(c, wi, wj)
    t = x.reshape(C, NS, WS, NS, WS)
    t = np.transpose(t, (1, 3, 0, 2, 4))
    return np.ascontiguousarray(t.reshape(S, D))


def _unpatchify(tokens):
    # (S, D) -> (1, C, IMG, IMG)
    t = tokens.reshape(NS, NS, C, WS, WS)
    t = np.transpose(t, (2, 0, 3, 1, 4))
    return np.ascontiguousarray(t.reshape(1, C, IMG, IMG))


def _prepare(inputs):
    x = np.asarray(inputs["x"], dtype=np.float32)
    Wq = np.asarray(inputs["Wq"], dtype=np.float32)
    Wk = np.asarray(inputs["Wk"], dtype=np.float32)
    Wv = np.asarray(inputs["Wv"], dtype=np.float32)
    bq = np.asarray(inputs["bq"], dtype=np.float32)
    bk = np.asarray(inputs["bk"], dtype=np.float32)
    bv = np.asarray(inputs["bv"], dtype=np.float32)
    gamma = np.asarray(inputs["gamma"], dtype=np.float32)
    beta = np.asarray(inputs["beta"], dtype=np.float32)

    general_gb = not (np.all(gamma == 1.0) and np.all(beta == 0.0))
    nc = _get_program(general_gb)

    dt = np.float16 if not general_gb else ml_dtypes.bfloat16
    xs = _patchify(x)

    def centered(W, b):
        Wc = W - W.mean(axis=0, keepdims=True)
        bc = b - b.mean()
        return Wc, bc

    Wqc, bqc = centered(Wq, bq)
    Wkc, bkc = centered(Wk, bk)
    Wvc, bvc = centered(Wv, bv)

    wa = np.concatenate([Wqc.T[0:128], Wkc.T[0:128], Wvc.T[0:128]], axis=1)
    wb = np.zeros((128, 3 * D), np.float32)
    wb[0:64, 0:D] = Wqc.T[128:192]
    wb[0:64, D:2 * D] = Wkc.T[128:192]
    wb[0:64, 2 * D:3 * D] = Wvc.T[128:192]
    wb[64, 0:D] = bqc
    wb[64, D:2 * D] = bkc
    wb[64, 2 * D:3 * D] = bvc
    wa = wa.astype(dt)
    wb = wb.astype(dt)
    wka = Wkc.T[0:128].astype(dt)
    wkb = np.zeros((128, D), np.float32)
    wkb[0:64] = Wkc.T[128:192]
    wkb[64] = bkc
    wkb = wkb.astype(dt)
    idn = np.eye(128, dtype=dt)

    dithb = (-SHIFT - np.log(_WCOL)).reshape(128, 1).astype(np.float32)
    dithw = _WCOL.reshape(128, 1).astype(np.float32)
    ones8 = _WCOL.reshape(128, 1).astype(ml_dtypes.float8_e4m3)

    in_maps = []
    for core in range(NCORES):
        g, h = core // 2, core % 2
        seg = xs[g * SEG:(g + 1) * SEG]
        perm = np.concatenate(
            [seg[h * NQ:(h + 1) * NQ], seg[(1 - h) * NQ:(2 - h) * NQ]],
            axis=0)
        xsT = perm.T  # (192, 4096)
        xav = np.ascontiguousarray(xsT[0:128]).astype(dt)
        xbv = np.zeros((128, SEG), np.float32)
        xbv[0:64] = xsT[128:192]
        xbv[64] = 1.0
        xbv = xbv.astype(dt)
        im = {"xa": xav, "xb": xbv, "wa": wa, "wb": wb,
              "wka": wka, "wkb": wkb, "idn": idn}
        if general_gb:
            im["gcol"] = gamma.reshape(D, 1).copy()
            im["bcol"] = beta.reshape(D, 1).copy()
            im["gbc"] = np.broadcast_to(gamma, (128, D)).copy()
            im["bbc"] = np.broadcast_to(beta, (128, D)).copy()
        else:
            im["dithb"] = dithb
            im["dithw"] = dithw
            im["ones8"] = ones8
        in_maps.append(im)

    return nc, in_maps, general_gb


def _postprocess(res):
    out_tokens = np.empty((S, D), np.float32)
    for core in range(NCORES):
        g, h = core // 2, core % 2
        outa = res.results[core]["outa"]  # (128, NQ) unnormalized outT
        outb = res.results[core]["outb"]  # (65, NQ): 0:64 outT, row 64 sums
        o_t = np.concatenate([outa, outb[0:64]], axis=0)  # (192, NQ)
        sums = outb[64]
        out_tokens[g * SEG + h * NQ: g * SEG + (h + 1) * NQ] = \
            (o_t / sums).T

    return _unpatchify(out_tokens)


def kernel(**inputs):
    nc, in_maps, _ = _prepare(inputs)
    res = run_bass_kernel_spmd(nc, in_maps, list(range(NCORES)))
    return _postprocess(res)
